# revision 1
# baseline (speedup 1.0000x reference)
"""BitMambaBlock Trainium2 kernel — 8-core SPMD.

Sharding: phase 1 (in_proj + conv + SSD scan) is head-sharded (8 heads/core);
phase 2 (out_proj) is token-sharded after an on-chip AllToAll reshard of y.

Numerics: both bitlinear matmuls run on the tensor engine in bf16 with EXACT
integer operands (activation quant produces ints in [-127,127]; weight quant
is ternary {-1,0,1}; both exact in bf16, fp32 PSUM accumulation, |sums| <
2^24, so the matmuls are bit-exact). The SSD scan uses the chunked (segsum)
formulation: chunk 128, intra-chunk via masked exp of cumsum differences,
inter-chunk via carried (N,P) states per head.
"""

import sys
import types
import numpy as np
import ml_dtypes

for _p in ("/opt/trn_rl_repo", "/root/.axon_site/_ro/trn_rl_repo"):
    if _p not in sys.path:
        sys.path.insert(0, _p)

# antenv.axon_hooks is missing from this image; bass_utils imports it when
# trace=True. Provide it (and register the NTFF hook when available).
try:
    import antenv

    if "antenv.axon_hooks" not in sys.modules:
        _mod = types.ModuleType("antenv.axon_hooks")
        _HOOK = [None]
        _mod.set_axon_ntff_profile_hook = lambda h: _HOOK.__setitem__(0, h)
        _mod.get_axon_ntff_profile_hook = lambda: _HOOK[0]
        sys.modules["antenv.axon_hooks"] = _mod
        antenv.axon_hooks = _mod
        try:
            from trn_agent_boot.trn_boot import _ntff_profile_via_ctypes

            _mod.set_axon_ntff_profile_hook(
                _ntff_profile_via_ctypes("/opt/axon/libaxon_pjrt.so")
            )
        except Exception:
            pass
except Exception:
    pass

import concourse.bass as bass
import concourse.bacc as bacc
import concourse.mybir as mybir
from concourse.tile import TileContext
from concourse import bass_utils

F32 = mybir.dt.float32
BF16 = mybir.dt.bfloat16
AF = mybir.ActivationFunctionType
ALU = mybir.AluOpType
AX = mybir.AxisListType

DIM = 2048
D_STATE = 128
D_CONV = 4
HEADDIM = 64
D_INNER = 4096
NHEADS = 64
D_IN_PROJ = 8512
CONV_DIM = 4352
BB, L = 2, 2048
NTOK = BB * L              # 4096
NCORES = 8
HPC = NHEADS // NCORES     # 8 heads per core
CPC = HPC * HEADDIM        # 512 d_inner channels per core
TPC = NTOK // NCORES       # 512 tokens per core (phase 2)
TC = 256                   # phase-1 time-chunk
Q = 128                    # scan chunk
MAGIC = float(np.float32(12582912.0))
NK1 = DIM // 128           # 16 k-tiles for in_proj
W1COLS = 1408              # x(512) B(128) C(128) dtpad(128) z(512)
NK2 = D_INNER // 128       # 32 k-tiles for out_proj

DEBUG_TAPS = False


def bcast_heads(ap, nh, rep):
    """[P, nh] -> [P, nh*rep] free-broadcast: each column repeated rep times."""
    return bass.AP(tensor=ap.tensor, offset=ap.offset,
                   ap=[list(ap.ap[0]), [ap.ap[1][0], nh], [0, rep]])


def _ln_stats_quant(nc, pool, H, scr, D, invs_dst, tagp):
    """LayerNorm stats + activation quant of H [128, D] (token-major).

    Writes inv_s (=1/s) to invs_dst [128,1]; returns qb [128, D] bf16 with
    integer values round((H - m) * rv * s). Uses accum sums (sh from caller),
    shh/max/min computed here. Caller already computed sh via its stt op.
    """
    raise NotImplementedError  # inlined below; kept for doc purposes


def build_bass(debug_taps=DEBUG_TAPS):
    nc = bacc.Bacc(None, target_bir_lowering=False, num_devices=NCORES)

    hid = nc.dram_tensor("hid", [NTOK, DIM], F32, kind="ExternalInput")
    hid2 = nc.dram_tensor("hid2", [TPC, DIM], F32, kind="ExternalInput")
    w1t = nc.dram_tensor("w1t", [DIM, W1COLS], BF16, kind="ExternalInput")
    w2t = nc.dram_tensor("w2t", [D_INNER, DIM], BF16, kind="ExternalInput")
    nwb = nc.dram_tensor("nwb", [128, DIM], F32, kind="ExternalInput")
    onwb = nc.dram_tensor("onwb", [128, D_INNER], F32, kind="ExternalInput")
    cw = nc.dram_tensor("cw", [768, D_CONV], F32, kind="ExternalInput")
    cb = nc.dram_tensor("cb", [768, 1], F32, kind="ExternalInput")
    dtb = nc.dram_tensor("dtb", [HPC, 1], F32, kind="ExternalInput")
    acoef = nc.dram_tensor("acoef", [HPC, 1], F32, kind="ExternalInput")
    drow = nc.dram_tensor("drow", [1, CPC], F32, kind="ExternalInput")
    maskneg = nc.dram_tensor("maskneg", [128, 128], F32, kind="ExternalInput")
    ident = nc.dram_tensor("ident", [128, 128], F32, kind="ExternalInput")

    out = nc.dram_tensor("out", [TPC, DIM], F32, kind="ExternalOutput")
    taps = {}
    if debug_taps:
        taps["tap_z"] = nc.dram_tensor("tap_z", [NTOK, CPC], F32, kind="ExternalOutput")
        taps["tap_conv"] = nc.dram_tensor("tap_conv", [768, NTOK], F32, kind="ExternalOutput")
        taps["tap_dt"] = nc.dram_tensor("tap_dt", [HPC, NTOK], F32, kind="ExternalOutput")
        taps["tap_y"] = nc.dram_tensor("tap_y", [NTOK, CPC], F32, kind="ExternalOutput")

    with TileContext(nc) as tc:
        with (
            tc.tile_pool(name="const", bufs=1) as constp,
            tc.tile_pool(name="dram", bufs=1, space="DRAM") as dram,
        ):
            a2a_in = dram.tile([NTOK, CPC], F32)
            a2a_out = dram.tile([NTOK, CPC], F32)

            # ---- resident constants ----
            W1S = constp.tile([128, NK1 * W1COLS], BF16)
            for kk in range(NK1):
                nc.sync.dma_start(W1S[:, kk * W1COLS:(kk + 1) * W1COLS],
                                  w1t[kk * 128:(kk + 1) * 128, :])
            NW = constp.tile([128, DIM], F32)
            nc.sync.dma_start(NW[:], nwb[:])
            CW = constp.tile([128, 6 * D_CONV], F32)
            CBt = constp.tile([128, 6], F32)
            for ct in range(6):
                nc.sync.dma_start(CW[:, ct * D_CONV:(ct + 1) * D_CONV],
                                  cw[ct * 128:(ct + 1) * 128, :])
                nc.sync.dma_start(CBt[:, ct:ct + 1], cb[ct * 128:(ct + 1) * 128, :])
            DTB = constp.tile([HPC, 1], F32)
            nc.sync.dma_start(DTB[:], dtb[:])
            ACO = constp.tile([HPC, 1], F32)
            nc.sync.dma_start(ACO[:], acoef[:])
            MASKN = constp.tile([128, 128], F32)
            nc.sync.dma_start(MASKN[:], maskneg[:])
            IDENT = constp.tile([128, 128], F32)
            nc.sync.dma_start(IDENT[:], ident[:])
            ONES1 = constp.tile([1, 128], F32)
            nc.vector.memset(ONES1[:], 1.0)
            DROW = constp.tile([1, CPC], F32)
            nc.sync.dma_start(DROW[:], drow[:])
            Z8 = constp.tile([HPC, Q], F32)
            nc.vector.memset(Z8[:], 0.0)
            EPS6 = constp.tile([128, 1], F32)
            nc.vector.memset(EPS6[:], 1e-6)
            EPS5 = constp.tile([128, 1], F32)
            nc.vector.memset(EPS5[:], 1e-5)
            hstate = constp.tile([128, CPC], F32, name="hstate")
            hstate_bf = constp.tile([128, CPC], BF16, name="hstate_bf")

            with tc.tile_pool(name="pc", bufs=1, space="PSUM") as pcp:
                DBCp = pcp.tile([128, CPC], F32)
                nc.tensor.matmul(DBCp[:], ONES1[:], DROW[:])
                DBC = constp.tile([128, CPC], F32)
                nc.vector.tensor_copy(DBC[:], DBCp[:])

            # ---- phase 1 ----
            with (
                tc.tile_pool(name="p1", bufs=1) as p1,
                tc.tile_pool(name="p1b", bufs=2) as p1b,
                tc.tile_pool(name="pmA", bufs=2, space="PSUM") as pmA,   # [128,512]
                tc.tile_pool(name="pmG", bufs=1, space="PSUM") as pmG,   # G
                tc.tile_pool(name="pmT", bufs=2, space="PSUM") as pmT,   # [128,128] misc
                tc.tile_pool(name="pmS", bufs=3, space="PSUM") as pmS,   # [128,64]
            ):
                for b in range(BB):
                    nc.vector.memset(hstate[:], 0.0)
                    nc.vector.memset(hstate_bf[:], 0.0)
                    xbcbuf_prev = None
                    for tcn in range(L // TC):
                        tok0 = b * L + tcn * TC
                        # ---- A: norms + quant + transpose ----
                        qT = p1b.tile([128, NK1 * TC], BF16, tag="qT")
                        isrow = p1b.tile([1, TC], F32, tag="isrow")
                        invs_cols = p1b.tile([128, TC // 128], F32, tag="invs")
                        for tt in range(TC // 128):
                            Xin = p1.tile([128, DIM], F32, tag="Xin", bufs=2)
                            nc.sync.dma_start(Xin[:], hid[tok0 + tt * 128: tok0 + (tt + 1) * 128, :])
                            scr = p1.tile([128, DIM], BF16, tag="scr")
                            ssq = p1.tile([128, 1], F32, tag="st1", bufs=2)
                            nc.scalar.activation(scr[:], Xin[:], AF.Square, accum_out=ssq[:])
                            lr = p1.tile([128, 1], F32, tag="st2", bufs=2)
                            nc.scalar.activation(lr[:], ssq[:], AF.Ln, bias=EPS6[:], scale=1.0 / DIM)
                            r = p1.tile([128, 1], F32, tag="st3", bufs=2)
                            nc.scalar.activation(r[:], lr[:], AF.Exp, scale=-0.5)
                            H = p1.tile([128, DIM], F32, tag="H", bufs=2)
                            sh = p1.tile([128, 1], F32, tag="st4", bufs=2)
                            nc.vector.scalar_tensor_tensor(
                                H[:], Xin[:], r[:], NW[:], ALU.mult, ALU.mult,
                                accum_out=sh[:])
                            shh = p1.tile([128, 1], F32, tag="st5", bufs=2)
                            nc.scalar.activation(scr[:], H[:], AF.Square, accum_out=shh[:])
                            hmax = p1.tile([128, 1], F32, tag="st6", bufs=2)
                            hmin = p1.tile([128, 1], F32, tag="st7", bufs=2)
                            nc.vector.tensor_reduce(out=hmax[:], in_=H[:], op=ALU.max, axis=AX.X)
                            nc.vector.tensor_reduce(out=hmin[:], in_=H[:], op=ALU.min, axis=AX.X)
                            m = p1.tile([128, 1], F32, tag="st8", bufs=2)
                            nc.vector.tensor_scalar(m[:], sh[:], 1.0 / DIM, None, ALU.mult)
                            m2n = p1.tile([128, 1], F32, tag="st9", bufs=2)
                            nc.vector.tensor_scalar(m2n[:], m[:], m[:], -1.0, ALU.mult, ALU.mult)
                            var = p1.tile([128, 1], F32, tag="st10", bufs=2)
                            nc.vector.scalar_tensor_tensor(var[:], shh[:], 1.0 / DIM, m2n[:], ALU.mult, ALU.add)
                            lv = p1.tile([128, 1], F32, tag="st11", bufs=2)
                            nc.scalar.activation(lv[:], var[:], AF.Ln, bias=EPS5[:])
                            rv = p1.tile([128, 1], F32, tag="st12", bufs=2)
                            nc.scalar.activation(rv[:], lv[:], AF.Exp, scale=-0.5)
                            d1 = p1.tile([128, 1], F32, tag="st13", bufs=2)
                            nc.vector.tensor_scalar(d1[:], hmax[:], m[:], None, ALU.subtract)
                            d2 = p1.tile([128, 1], F32, tag="st14", bufs=2)
                            nc.vector.tensor_scalar(d2[:], hmin[:], -1.0, m[:], ALU.mult, ALU.add)
                            mab = p1.tile([128, 1], F32, tag="st15", bufs=2)
                            nc.vector.tensor_tensor(mab[:], d1[:], d2[:], ALU.max)
                            t1 = p1.tile([128, 1], F32, tag="st16", bufs=2)
                            nc.vector.tensor_tensor(t1[:], rv[:], mab[:], ALU.mult)
                            t2 = p1.tile([128, 1], F32, tag="st17", bufs=2)
                            nc.vector.tensor_scalar(t2[:], t1[:], 1e-5, None, ALU.max)
                            nc.vector.tensor_scalar(invs_cols[:, tt:tt + 1], t2[:], 1.0 / 127.0, None, ALU.mult)
                            rt2 = p1.tile([128, 1], F32, tag="st18", bufs=2)
                            nc.vector.reciprocal(rt2[:], t2[:])
                            scl = p1.tile([128, 1], F32, tag="st19", bufs=2)
                            nc.vector.tensor_scalar(scl[:], rt2[:], 127.0, rv[:], ALU.mult, ALU.mult)
                            mscl = p1.tile([128, 1], F32, tag="st20", bufs=2)
                            nc.vector.tensor_tensor(mscl[:], m[:], scl[:], ALU.mult)
                            b2 = p1.tile([128, 1], F32, tag="st21", bufs=2)
                            nc.vector.tensor_scalar(b2[:], mscl[:], -1.0, None, ALU.mult)
                            T1 = p1.tile([128, DIM], F32, tag="Xin", bufs=2, name="T1")
                            nc.vector.tensor_scalar(T1[:], H[:], scl[:], b2[:], ALU.mult, ALU.add)
                            qb = p1.tile([128, DIM], BF16, tag="qb")
                            nc.vector.tensor_scalar(qb[:], T1[:], MAGIC, MAGIC, ALU.add, ALU.subtract)
                            for dd in range(NK1):
                                nc.sync.dma_start_transpose(
                                    qT[:, dd * TC + tt * 128: dd * TC + (tt + 1) * 128],
                                    qb[:, dd * 128:(dd + 1) * 128])
                            nc.sync.dma_start(isrow[0:1, tt * 128:(tt + 1) * 128],
                                              invs_cols[:, tt:tt + 1])
                        # s_bcast [128, TC]
                        sbp = pmA.tile([128, TC], F32, tag="m512")
                        nc.tensor.matmul(sbp[:], ONES1[:], isrow[:])
                        SB = p1b.tile([128, TC], F32, tag="SB")
                        nc.vector.tensor_copy(SB[:], sbp[:])

                        # ---- C: z matmuls (token-major) + silu ----
                        sz = p1b.tile([128, (TC // 128) * CPC], F32, tag="sz")
                        for tt in range(TC // 128):
                            pz = pmA.tile([128, CPC], F32, tag="m512")
                            for kk in range(NK1):
                                nc.tensor.matmul(
                                    pz[:],
                                    qT[:, kk * TC + tt * 128: kk * TC + (tt + 1) * 128],
                                    W1S[:, kk * W1COLS + 896: (kk + 1) * W1COLS],
                                    start=(kk == 0), stop=(kk == NK1 - 1))
                            nc.scalar.activation(sz[:, tt * CPC:(tt + 1) * CPC], pz[:],
                                                 AF.Silu, scale=invs_cols[:, tt:tt + 1])
                            if debug_taps:
                                zt = p1.tile([128, CPC], F32, tag="ztap")
                                nc.scalar.activation(zt[:], pz[:], AF.Copy,
                                                     scale=invs_cols[:, tt:tt + 1])
                                nc.sync.dma_start(
                                    taps["tap_z"][tok0 + tt * 128: tok0 + (tt + 1) * 128, :], zt[:])

                        # ---- D: xBC + dt matmuls (channel-major) ----
                        xbcbuf = p1b.tile([128, 6 * (TC + 3)], F32, tag="xbcbuf")
                        dtraw = p1b.tile([HPC, TC], F32, tag="dtraw")
                        for cbk in range(7):
                            px = pmA.tile([128, TC], F32, tag="m512")
                            for kk in range(NK1):
                                nc.tensor.matmul(
                                    px[:],
                                    W1S[:, kk * W1COLS + cbk * 128: kk * W1COLS + (cbk + 1) * 128],
                                    qT[:, kk * TC:(kk + 1) * TC],
                                    start=(kk == 0), stop=(kk == NK1 - 1))
                            if cbk < 6:
                                dst = xbcbuf[:, cbk * (TC + 3) + 3: (cbk + 1) * (TC + 3)]
                                nc.vector.tensor_tensor(dst, px[:], SB[:], ALU.mult)
                            else:
                                nc.vector.tensor_tensor(dtraw[:], px[0:HPC, :], SB[0:HPC, :], ALU.mult)
                        # halo: first 3 cols
                        for ct in range(6):
                            h0 = xbcbuf[:, ct * (TC + 3): ct * (TC + 3) + 3]
                            if tcn == 0:
                                nc.vector.memset(h0, 0.0)
                            else:
                                nc.vector.tensor_copy(
                                    h0, xbcbuf_prev[:, ct * (TC + 3) + TC: (ct + 1) * (TC + 3)])
                        xbcbuf_prev = xbcbuf

                        # ---- E: conv + silu + casts ----
                        xbf = p1b.tile([128, 4 * TC], BF16, tag="xbf")
                        xf32 = p1b.tile([128, 4 * TC], F32, tag="xf32")
                        bbf = p1b.tile([128, TC], BF16, tag="bbf")
                        cbf = p1b.tile([128, TC], BF16, tag="cbf")
                        for ct in range(6):
                            conv = p1.tile([128, TC], F32, tag="conv", bufs=2)
                            base = ct * (TC + 3)
                            nc.vector.tensor_scalar(conv[:], xbcbuf[:, base: base + TC],
                                                    CW[:, ct * D_CONV: ct * D_CONV + 1], None, ALU.mult)
                            for k in range(1, D_CONV):
                                nc.vector.scalar_tensor_tensor(
                                    conv[:], xbcbuf[:, base + k: base + k + TC],
                                    CW[:, ct * D_CONV + k: ct * D_CONV + k + 1],
                                    conv[:], ALU.mult, ALU.add)
                            cs = p1.tile([128, TC], F32, tag="cs", bufs=2)
                            nc.scalar.activation(cs[:], conv[:], AF.Silu, bias=CBt[:, ct:ct + 1])
                            if debug_taps:
                                nc.sync.dma_start(
                                    taps["tap_conv"][ct * 128:(ct + 1) * 128,
                                                     b * L + tcn * TC: b * L + (tcn + 1) * TC], cs[:])
                            if ct < 4:
                                nc.vector.tensor_copy(xbf[:, ct * TC:(ct + 1) * TC], cs[:])
                                nc.vector.tensor_copy(xf32[:, ct * TC:(ct + 1) * TC], cs[:])
                            elif ct == 4:
                                nc.vector.tensor_copy(bbf[:], cs[:])
                            else:
                                nc.vector.tensor_copy(cbf[:], cs[:])

                        # ---- F: dt pipeline (softplus composed) ----
                        dts = p1b.tile([HPC, TC], F32, tag="dts")
                        t_ab = p1.tile([HPC, TC], F32, tag="dta")
                        nc.scalar.activation(t_ab[:], dtraw[:], AF.Abs, bias=DTB[:])
                        t_e = p1.tile([HPC, TC], F32, tag="dte")
                        nc.scalar.activation(t_e[:], t_ab[:], AF.Exp, scale=-1.0)
                        t_l = p1.tile([HPC, TC], F32, tag="dtl")
                        nc.scalar.activation(t_l[:], t_e[:], AF.Ln, bias=1.0)
                        t_r = p1.tile([HPC, TC], F32, tag="dtr")
                        nc.scalar.activation(t_r[:], dtraw[:], AF.Relu, bias=DTB[:])
                        nc.vector.tensor_tensor(dts[:], t_l[:], t_r[:], ALU.add)
                        if debug_taps:
                            nc.sync.dma_start(
                                taps["tap_dt"][:, b * L + tcn * TC: b * L + (tcn + 1) * TC], dts[:])
                        av = p1b.tile([HPC, TC], F32, tag="av")
                        nc.vector.tensor_scalar(av[:], dts[:], ACO[:], None, ALU.mult)

                        # ---- G: scan chunks ----
                        for cq in range(TC // Q):
                            csl = slice(cq * Q, (cq + 1) * Q)
                            S = p1.tile([HPC, Q], F32, tag="S", bufs=2)
                            nc.vector.tensor_tensor_scan(S[:], av[:, csl], Z8[:], 0.0, ALU.add, ALU.add)
                            T = p1.tile([HPC, Q], F32, tag="T", bufs=2)
                            nc.vector.tensor_scalar(T[:], S[:], -1.0, S[:, Q - 1:Q], ALU.mult, ALU.add)
                            pst = pmT.tile([128, HPC], F32, tag="t128")
                            nc.tensor.transpose(pst[:], S[:], IDENT[0:HPC, 0:HPC])
                            ST = p1.tile([128, HPC], F32, tag="ST", bufs=2)
                            nc.vector.tensor_copy(ST[:], pst[:])
                            pet = pmT.tile([128, HPC], F32, tag="t128")
                            nc.tensor.transpose(pet[:], T[:], IDENT[0:HPC, 0:HPC])
                            eT = p1.tile([128, HPC], F32, tag="eT", bufs=2)
                            nc.scalar.activation(eT[:], pet[:], AF.Exp)
                            eST = p1.tile([128, HPC], F32, tag="eST", bufs=2)
                            nc.scalar.activation(eST[:], ST[:], AF.Exp)
                            pdt = pmT.tile([128, HPC], F32, tag="t128")
                            nc.tensor.transpose(pdt[:], dts[:, csl], IDENT[0:HPC, 0:HPC])
                            dtsT = p1.tile([128, HPC], F32, tag="dtsT", bufs=2)
                            nc.vector.tensor_copy(dtsT[:], pdt[:])
                            eSQ = p1.tile([128, HPC], F32, tag="eSQ", bufs=2)
                            nc.vector.tensor_tensor(eSQ[:], eT[:], eST[:], ALU.mult)
                            # f32 dt broadcast (intra path), bf16 dt*exp broadcast (state path)
                            dtbcf = p1.tile([128, CPC], F32, tag="dtbcf", bufs=2)
                            nc.vector.tensor_copy(dtbcf[:], bcast_heads(dtsT[:, 0:HPC], HPC, HEADDIM))
                            de = p1.tile([128, HPC], F32, tag="de", bufs=2)
                            nc.vector.tensor_tensor(de[:], dtsT[:], eT[:], ALU.mult)
                            debc = p1.tile([128, CPC], BF16, tag="debc", bufs=2)
                            nc.vector.tensor_copy(debc[:], bcast_heads(de[:, 0:HPC], HPC, HEADDIM))
                            x_tm = p1.tile([128, CPC], BF16, tag="x_tm", bufs=2)
                            for ct in range(4):
                                nc.sync.dma_start_transpose(
                                    x_tm[:, ct * 128:(ct + 1) * 128],
                                    xbf[:, ct * TC + cq * Q: ct * TC + (cq + 1) * Q])
                            # f32 x^T via PE transpose, fused with dt scale -> X2f
                            X2f = p1.tile([128, CPC], F32, tag="X2f", bufs=2)
                            for ct in range(4):
                                pxt = pmT.tile([128, 128], F32, tag="t128", name=f"pxt{ct}")
                                nc.tensor.transpose(pxt[:], xf32[:, ct * TC + cq * Q: ct * TC + (cq + 1) * Q], IDENT[:])
                                nc.vector.tensor_tensor(X2f[:, ct * 128:(ct + 1) * 128], pxt[:],
                                                        dtbcf[:, ct * 128:(ct + 1) * 128], ALU.mult)
                            X3 = p1.tile([128, CPC], BF16, tag="X3", bufs=2)
                            nc.vector.tensor_tensor(X3[:], x_tm[:], debc[:], ALU.mult)
                            btm = p1.tile([128, 128], BF16, tag="btm", bufs=2)
                            nc.sync.dma_start_transpose(btm[:], bbf[:, csl])
                            pg = pmG.tile([128, 128], F32, tag="pg")
                            nc.tensor.matmul(pg[:], bbf[:, csl], cbf[:, csl])
                            y_sb = p1.tile([128, CPC], F32, tag="y_sb", bufs=2)
                            srow = p1.tile([1, HPC * Q], F32, tag="srow", bufs=2)
                            nc.sync.dma_start(srow[0:1, :], S[:])
                            for hh in range(HPC):
                                hsl = slice(hh * HEADDIM, (hh + 1) * HEADDIM)
                                psb = pmT.tile([128, 128], F32, tag="t128")
                                nc.tensor.matmul(psb[:], ONES1[:], srow[0:1, hh * Q:(hh + 1) * Q])
                                Df = p1.tile([128, 128], F32, tag="Df", bufs=2)
                                nc.vector.tensor_scalar(Df[:], psb[:], ST[:, hh:hh + 1], None, ALU.subtract)
                                Dm = p1.tile([128, 128], F32, tag="Dm", bufs=2)
                                nc.vector.tensor_tensor(Dm[:], Df[:], MASKN[:], ALU.add)
                                Mx = p1.tile([128, 128], F32, tag="Mx", bufs=2)
                                nc.scalar.activation(Mx[:], Dm[:], AF.Exp)
                                Wm = p1.tile([128, 128], F32, tag="Wm", bufs=2)
                                nc.vector.tensor_tensor(Wm[:], pg[:], Mx[:], ALU.mult)
                                pyi = pmS.tile([128, HEADDIM], F32, tag="s64")
                                nc.tensor.matmul(pyi[:], Wm[:], X2f[:, hsl])
                                pye = pmS.tile([128, HEADDIM], F32, tag="s64")
                                nc.tensor.matmul(pye[:], cbf[:, csl], hstate_bf[:, hsl])
                                yi_sb = p1.tile([128, HEADDIM], F32, tag="yi_sb", bufs=3)
                                nc.vector.tensor_copy(yi_sb[:], pyi[:])
                                nc.vector.scalar_tensor_tensor(
                                    y_sb[:, hsl], pye[:], eST[:, hh:hh + 1], yi_sb[:],
                                    ALU.mult, ALU.add)
                                pd = pmS.tile([128, HEADDIM], F32, tag="s64")
                                nc.tensor.matmul(pd[:], btm[:], X3[:, hsl])
                                nc.vector.scalar_tensor_tensor(
                                    hstate[:, hsl], hstate[:, hsl], eSQ[:, hh:hh + 1], pd[:],
                                    ALU.mult, ALU.add)
                                nc.vector.tensor_copy(hstate_bf[:, hsl], hstate[:, hsl])
                            # y = (y + D*x) * silu(z)
                            dx = p1.tile([128, CPC], F32, tag="X2f", bufs=2, name="dx")
                            nc.vector.tensor_tensor(dx[:], x_tm[:], DBC[:], ALU.mult)
                            nc.vector.tensor_tensor(y_sb[:], y_sb[:], dx[:], ALU.add)
                            nc.vector.tensor_tensor(y_sb[:], y_sb[:], sz[:, cq * CPC:(cq + 1) * CPC], ALU.mult)
                            nc.sync.dma_start(a2a_in[tok0 + cq * Q: tok0 + (cq + 1) * Q, :], y_sb[:])
                            if debug_taps:
                                nc.sync.dma_start(taps["tap_y"][tok0 + cq * Q: tok0 + (cq + 1) * Q, :], y_sb[:])

            # ---- AllToAll ----
            nc.gpsimd.collective_compute(
                "AllToAll", ALU.bypass,
                replica_groups=[list(range(NCORES))],
                ins=[a2a_in[:]], outs=[a2a_out[:]])

            # ---- phase 2 ----
            with (
                tc.tile_pool(name="p2", bufs=1) as p2,
                tc.tile_pool(name="p2b", bufs=3) as p2b,
                tc.tile_pool(name="q2", bufs=1, space="PSUM") as pq2,
            ):
                ONW = p2.tile([128, D_INNER], F32, tag="ONW")
                nc.sync.dma_start(ONW[:], onwb[:])
                qT2 = p2.tile([128, NK2 * 512], BF16, tag="qT2")
                invs2 = p2.tile([128, 4], F32, tag="invs2")
                for tt in range(4):
                    Y2 = p2.tile([128, D_INNER], F32, tag="Y2")
                    for j in range(NCORES):
                        nc.sync.dma_start(
                            Y2[:, j * CPC:(j + 1) * CPC],
                            a2a_out[j * TPC + tt * 128: j * TPC + (tt + 1) * 128, :])
                    scr = p2.tile([128, D_INNER], BF16, tag="scr2")
                    ssq = p2.tile([128, 1], F32, tag="u1", bufs=2)
                    nc.scalar.activation(scr[:], Y2[:], AF.Square, accum_out=ssq[:])
                    lr = p2.tile([128, 1], F32, tag="u2", bufs=2)
                    nc.scalar.activation(lr[:], ssq[:], AF.Ln, bias=EPS6[:], scale=1.0 / D_INNER)
                    r = p2.tile([128, 1], F32, tag="u3", bufs=2)
                    nc.scalar.activation(r[:], lr[:], AF.Exp, scale=-0.5)
                    H = p2.tile([128, D_INNER], F32, tag="H2")
                    sh = p2.tile([128, 1], F32, tag="u4", bufs=2)
                    nc.vector.scalar_tensor_tensor(H[:], Y2[:], r[:], ONW[:], ALU.mult, ALU.mult, accum_out=sh[:])
                    shh = p2.tile([128, 1], F32, tag="u5", bufs=2)
                    nc.scalar.activation(scr[:], H[:], AF.Square, accum_out=shh[:])
                    hmax = p2.tile([128, 1], F32, tag="u6", bufs=2)
                    hmin = p2.tile([128, 1], F32, tag="u7", bufs=2)
                    nc.vector.tensor_reduce(out=hmax[:], in_=H[:], op=ALU.max, axis=AX.X)
                    nc.vector.tensor_reduce(out=hmin[:], in_=H[:], op=ALU.min, axis=AX.X)
                    m = p2.tile([128, 1], F32, tag="u8", bufs=2)
                    nc.vector.tensor_scalar(m[:], sh[:], 1.0 / D_INNER, None, ALU.mult)
                    m2n = p2.tile([128, 1], F32, tag="u9", bufs=2)
                    nc.vector.tensor_scalar(m2n[:], m[:], m[:], -1.0, ALU.mult, ALU.mult)
                    var = p2.tile([128, 1], F32, tag="u10", bufs=2)
                    nc.vector.scalar_tensor_tensor(var[:], shh[:], 1.0 / D_INNER, m2n[:], ALU.mult, ALU.add)
                    lv = p2.tile([128, 1], F32, tag="u11", bufs=2)
                    nc.scalar.activation(lv[:], var[:], AF.Ln, bias=EPS5[:])
                    rv = p2.tile([128, 1], F32, tag="u12", bufs=2)
                    nc.scalar.activation(rv[:], lv[:], AF.Exp, scale=-0.5)
                    d1 = p2.tile([128, 1], F32, tag="u13", bufs=2)
                    nc.vector.tensor_scalar(d1[:], hmax[:], m[:], None, ALU.subtract)
                    d2 = p2.tile([128, 1], F32, tag="u14", bufs=2)
                    nc.vector.tensor_scalar(d2[:], hmin[:], -1.0, m[:], ALU.mult, ALU.add)
                    mab = p2.tile([128, 1], F32, tag="u15", bufs=2)
                    nc.vector.tensor_tensor(mab[:], d1[:], d2[:], ALU.max)
                    t1 = p2.tile([128, 1], F32, tag="u16", bufs=2)
                    nc.vector.tensor_tensor(t1[:], rv[:], mab[:], ALU.mult)
                    t2 = p2.tile([128, 1], F32, tag="u17", bufs=2)
                    nc.vector.tensor_scalar(t2[:], t1[:], 1e-5, None, ALU.max)
                    nc.vector.tensor_scalar(invs2[:, tt:tt + 1], t2[:], 1.0 / 127.0, None, ALU.mult)
                    rt2 = p2.tile([128, 1], F32, tag="u18", bufs=2)
                    nc.vector.reciprocal(rt2[:], t2[:])
                    scl = p2.tile([128, 1], F32, tag="u19", bufs=2)
                    nc.vector.tensor_scalar(scl[:], rt2[:], 127.0, rv[:], ALU.mult, ALU.mult)
                    mscl = p2.tile([128, 1], F32, tag="u20", bufs=2)
                    nc.vector.tensor_tensor(mscl[:], m[:], scl[:], ALU.mult)
                    b2 = p2.tile([128, 1], F32, tag="u21", bufs=2)
                    nc.vector.tensor_scalar(b2[:], mscl[:], -1.0, None, ALU.mult)
                    T1 = p2.tile([128, D_INNER], F32, tag="Y2", name="T12")
                    nc.vector.tensor_scalar(T1[:], H[:], scl[:], b2[:], ALU.mult, ALU.add)
                    qb = p2.tile([128, D_INNER], BF16, tag="qb2")
                    nc.vector.tensor_scalar(qb[:], T1[:], MAGIC, MAGIC, ALU.add, ALU.subtract)
                    for dd in range(NK2):
                        nc.sync.dma_start_transpose(
                            qT2[:, dd * 512 + tt * 128: dd * 512 + (tt + 1) * 128],
                            qb[:, dd * 128:(dd + 1) * 128])

                # out_proj matmuls: out (t, c); k outer so each w2 tile loads once
                for cc in range(4):
                    pos = [pq2.tile([128, 512], F32, tag=f"po{t}", name=f"po{t}_{cc}") for t in range(4)]
                    for kk in range(NK2):
                        w2tile = p2b.tile([128, 512], BF16, tag="w2tile")
                        nc.sync.dma_start(w2tile[:], w2t[kk * 128:(kk + 1) * 128, cc * 512:(cc + 1) * 512])
                        for tt in range(4):
                            nc.tensor.matmul(
                                pos[tt][:],
                                qT2[:, kk * 512 + tt * 128: kk * 512 + (tt + 1) * 128],
                                w2tile[:],
                                start=(kk == 0), stop=(kk == NK2 - 1))
                    for tt in range(4):
                        hidt = p2b.tile([128, 512], F32, tag="hidt")
                        nc.sync.dma_start(hidt[:], hid2[tt * 128:(tt + 1) * 128, cc * 512:(cc + 1) * 512])
                        ot = p2b.tile([128, 512], F32, tag="ot")
                        nc.vector.scalar_tensor_tensor(
                            ot[:], pos[tt][:], invs2[:, tt:tt + 1], hidt[:], ALU.mult, ALU.add)
                        nc.sync.dma_start(out[tt * 128:(tt + 1) * 128, cc * 512:(cc + 1) * 512], ot[:])

    nc.compile()
    return nc


_CACHE = {}


def _prep_inputs(inputs):
    hid = np.ascontiguousarray(np.asarray(inputs["hidden_states"], np.float32).reshape(NTOK, DIM))
    w1 = np.asarray(inputs["in_proj_w"], np.float32)
    w2 = np.asarray(inputs["out_proj_w"], np.float32)

    def wquant(w):
        scale = max(np.float32(np.mean(np.abs(w), dtype=np.float32)), np.float32(1e-5))
        ws = w / scale
        return np.clip(np.where(ws >= 0, np.floor(ws + 0.5), np.ceil(ws - 0.5)), -1.0, 1.0).astype(np.float32)

    q1 = wquant(w1)
    q2 = wquant(w2)
    conv_w = np.asarray(inputs["conv_w"], np.float32)[:, 0, :]
    conv_b = np.asarray(inputs["conv_b"], np.float32)
    A = -np.exp(np.asarray(inputs["A_log"], np.float32))
    Dv = np.asarray(inputs["D"], np.float32)
    dtb = np.asarray(inputs["dt_bias"], np.float32)
    nw = np.asarray(inputs["norm_w"], np.float32)
    onw = np.asarray(inputs["out_norm_w"], np.float32)

    nwb = np.ascontiguousarray(np.broadcast_to(nw[None, :], (128, DIM)))
    onwb = np.ascontiguousarray(np.broadcast_to(onw[None, :], (128, D_INNER)))
    w2t = np.ascontiguousarray(q2.T.astype(ml_dtypes.bfloat16))
    ii, jj = np.meshgrid(np.arange(128), np.arange(128), indexing="ij")
    maskneg = np.where(ii > jj, np.float32(-1e30), np.float32(0.0)).astype(np.float32)
    identity = np.eye(128, dtype=np.float32)

    in_maps = []
    for k in range(NCORES):
        wk = np.concatenate([
            q1[D_INNER + CPC * k: D_INNER + CPC * (k + 1)],        # x
            q1[2 * D_INNER: 2 * D_INNER + D_STATE],                # B
            q1[2 * D_INNER + D_STATE: 2 * D_INNER + 2 * D_STATE],  # C
            np.concatenate([q1[2 * D_INNER + 2 * D_STATE + HPC * k:
                               2 * D_INNER + 2 * D_STATE + HPC * (k + 1)],
                            np.zeros((128 - HPC, DIM), np.float32)], axis=0),
            q1[CPC * k: CPC * (k + 1)],                            # z
        ], axis=0)
        w1tk = np.ascontiguousarray(wk.T.astype(ml_dtypes.bfloat16))
        cwk = np.concatenate([conv_w[CPC * k: CPC * (k + 1)],
                              conv_w[D_INNER: D_INNER + D_STATE],
                              conv_w[D_INNER + D_STATE:]], axis=0)
        cbk = np.concatenate([conv_b[CPC * k: CPC * (k + 1)],
                              conv_b[D_INNER: D_INNER + D_STATE],
                              conv_b[D_INNER + D_STATE:]])[:, None]
        in_maps.append({
            "hid": hid,
            "hid2": np.ascontiguousarray(hid[TPC * k: TPC * (k + 1)]),
            "w1t": w1tk,
            "w2t": w2t,
            "nwb": nwb,
            "onwb": onwb,
            "cw": np.ascontiguousarray(cwk),
            "cb": np.ascontiguousarray(cbk),
            "dtb": np.ascontiguousarray(dtb[HPC * k: HPC * (k + 1)][:, None]),
            "acoef": np.ascontiguousarray(A[HPC * k: HPC * (k + 1)][:, None]),
            "drow": np.ascontiguousarray(np.repeat(Dv[HPC * k: HPC * (k + 1)], HEADDIM)[None, :]),
            "maskneg": maskneg,
            "ident": identity,
        })
    return in_maps


def kernel(**inputs):
    if "nc" not in _CACHE:
        _CACHE["nc"] = build_bass()
    nc = _CACHE["nc"]
    in_maps = _prep_inputs(inputs)
    res = bass_utils.run_bass_kernel_spmd(nc, in_maps, core_ids=list(range(NCORES)))
    _CACHE["last_results"] = res
    outp = np.concatenate([res.results[k]["out"] for k in range(NCORES)], axis=0)
    return outp.reshape(BB, L, DIM).astype(np.float32)



# revision 22
# speedup vs baseline: 1.1761x; 1.1761x over previous
"""BitMambaBlock Trainium2 kernel — 8-core SPMD, v2.

Sharding: phase 1 (in_proj + conv + SSD scan) head-sharded (8 heads/core over
all 4096 tokens); phase 2 (out_proj) token-sharded (512 tokens/core, 256 from
each batch) after a 2-stage bf16 AllToAll (batch-0 a2a overlaps batch-1
phase-1 compute).

Numerics: bitlinear matmuls use exact integer bf16 operands (quant ints in
[-127,127], ternary weights) with fp32 PSUM accumulation. The rmsnorm before
each bitlinear folds away (layernorm_noaffine is invariant to per-token
positive scaling), so LN stats are computed directly on x*norm_w. SSD scan
uses the chunked masked-segsum form, chunk=128, bf16 matmul operands, fp32
state; dt and the D*x skip-term are folded into the intra-chunk weight matrix.
"""

import sys
import types
import numpy as np
import ml_dtypes

for _p in ("/opt/trn_rl_repo", "/root/.axon_site/_ro/trn_rl_repo"):
    if _p not in sys.path:
        sys.path.insert(0, _p)

try:
    import antenv

    if "antenv.axon_hooks" not in sys.modules:
        _mod = types.ModuleType("antenv.axon_hooks")
        _HOOK = [None]
        _mod.set_axon_ntff_profile_hook = lambda h: _HOOK.__setitem__(0, h)
        _mod.get_axon_ntff_profile_hook = lambda: _HOOK[0]
        sys.modules["antenv.axon_hooks"] = _mod
        antenv.axon_hooks = _mod
        try:
            from trn_agent_boot.trn_boot import _ntff_profile_via_ctypes

            _mod.set_axon_ntff_profile_hook(
                _ntff_profile_via_ctypes("/opt/axon/libaxon_pjrt.so")
            )
        except Exception:
            pass
except Exception:
    pass

import concourse.bass as bass
import concourse.bacc as bacc
import concourse.mybir as mybir
from concourse.tile import TileContext
from concourse import bass_utils

F32 = mybir.dt.float32
BF16 = mybir.dt.bfloat16
AF = mybir.ActivationFunctionType
ALU = mybir.AluOpType
AX = mybir.AxisListType

DIM = 2048
D_STATE = 128
D_CONV = 4
HEADDIM = 64
D_INNER = 4096
NHEADS = 64
D_IN_PROJ = 8512
CONV_DIM = 4352
BB, L = 2, 2048
NTOK = BB * L              # 4096
NCORES = 8
HPC = NHEADS // NCORES     # 8 heads per core
CPC = HPC * HEADDIM        # 512 d_inner channels per core
TPC = NTOK // NCORES       # 512 tokens per core (phase 2)
HTOK = NTOK // 2           # 2048 tokens per batch
TPH = TPC // 2             # 256 tokens per core per half
TC = 512                   # phase-1 block
NSUB = TC // 128           # 4 subtiles per block
Q = 128                    # scan chunk
MAGIC = float(np.float32(12582912.0))
NK1 = DIM // 128           # 16 k-tiles for in_proj
W1COLS = 1408              # x(512) B(128) C(128) dtpad(128) z(512); z at 896
NK2 = D_INNER // 128       # 32 k-tiles for out_proj

QT_ON_PE = False           # qb transposes: False=DMA queues, True=TensorE
DEBUG_TAPS = False


def bcast(ap, n_outer, stride_outer, rep):
    """[P, n_outer(+)] -> [P, n_outer, rep] view (each col repeated rep x)."""
    return bass.AP(tensor=ap.tensor, offset=ap.offset,
                   ap=[list(ap.ap[0]), [stride_outer, n_outer], [0, rep]])


def block_rep(ap, nrep, ncols):
    """[P, ncols] -> [P, nrep, ncols] view (whole block repeated nrep x)."""
    st = ap.ap[1][0]
    return bass.AP(tensor=ap.tensor, offset=ap.offset,
                   ap=[list(ap.ap[0]), [0, nrep], [st, ncols]])


def row_view(tile_ap, n_outer, inner):
    """[n_outer, inner] partition-major tile viewed as [1, n_outer, inner]."""
    return bass.AP(tensor=tile_ap.tensor, offset=tile_ap.offset,
                   ap=[[tile_ap.ap[0][0], 1], [inner, n_outer], [1, inner]])


def build_bass():
    nc = bacc.Bacc(None, target_bir_lowering=False, num_devices=NCORES)

    hid = nc.dram_tensor("hid", [NTOK, DIM], F32, kind="ExternalInput")
    hid2 = nc.dram_tensor("hid2", [TPC, DIM], F32, kind="ExternalInput")
    w1t = nc.dram_tensor("w1t", [DIM, W1COLS], BF16, kind="ExternalInput")
    w2t = nc.dram_tensor("w2t", [D_INNER, DIM], BF16, kind="ExternalInput")
    nwb = nc.dram_tensor("nwb", [128, DIM], F32, kind="ExternalInput")
    onwb = nc.dram_tensor("onwb", [128, D_INNER], F32, kind="ExternalInput")
    cw = nc.dram_tensor("cw", [768, D_CONV], F32, kind="ExternalInput")
    cb = nc.dram_tensor("cb", [768, 1], F32, kind="ExternalInput")
    dtb = nc.dram_tensor("dtb", [HPC, 1], F32, kind="ExternalInput")
    acoef = nc.dram_tensor("acoef", [HPC, 1], F32, kind="ExternalInput")
    maskt = nc.dram_tensor("maskt", [128, 512], BF16, kind="ExternalInput")
    drow = nc.dram_tensor("drow", [1, CPC], F32, kind="ExternalInput")
    ident = nc.dram_tensor("ident", [128, 128], F32, kind="ExternalInput")
    identb = nc.dram_tensor("identb", [128, 128], BF16, kind="ExternalInput")

    out = nc.dram_tensor("out", [TPC, DIM], F32, kind="ExternalOutput")
    taps = {}
    if DEBUG_TAPS:
        taps["tap_z"] = nc.dram_tensor("tap_z", [NTOK, CPC], BF16, kind="ExternalOutput")
        taps["tap_conv"] = nc.dram_tensor("tap_conv", [768, NTOK], BF16, kind="ExternalOutput")
        taps["tap_dt"] = nc.dram_tensor("tap_dt", [HPC, NTOK], F32, kind="ExternalOutput")
        taps["tap_y"] = nc.dram_tensor("tap_y", [NTOK, CPC], BF16, kind="ExternalOutput")
        taps["tap_xbc"] = nc.dram_tensor("tap_xbc", [768, NTOK], BF16, kind="ExternalOutput")

    with TileContext(nc) as tc:
        with (
            tc.tile_pool(name="const", bufs=1) as constp,
            tc.tile_pool(name="dram", bufs=1, space="DRAM") as dram,
        ):
            a2a_in = [dram.tile([HTOK, CPC], BF16, name=f"a2ai{i}") for i in (0, 1)]
            a2a_out = [dram.tile([HTOK, CPC], BF16, name=f"a2ao{i}") for i in (0, 1)]

            # ---- resident constants ----
            W1S = constp.tile([128, NK1 * W1COLS], BF16)
            for kk in range(NK1):
                nc.sync.dma_start(W1S[:, kk * W1COLS:(kk + 1) * W1COLS],
                                  w1t[kk * 128:(kk + 1) * 128, :])
            NW = constp.tile([128, DIM], F32)
            nc.sync.dma_start(NW[:], nwb[:])
            CW = constp.tile([128, 6 * D_CONV], F32)
            CBt = constp.tile([128, 6], F32)
            for ct in range(6):
                nc.sync.dma_start(CW[:, ct * D_CONV:(ct + 1) * D_CONV],
                                  cw[ct * 128:(ct + 1) * 128, :])
                nc.sync.dma_start(CBt[:, ct:ct + 1], cb[ct * 128:(ct + 1) * 128, :])
            DTB = constp.tile([HPC, 1], F32)
            nc.sync.dma_start(DTB[:], dtb[:])
            ACO = constp.tile([HPC, 1], F32)
            nc.sync.dma_start(ACO[:], acoef[:])
            MASKT = constp.tile([128, 512], BF16)
            nc.sync.dma_start(MASKT[:], maskt[:])
            DROW = constp.tile([1, CPC], F32)
            nc.sync.dma_start(DROW[:], drow[:])
            IDENT = constp.tile([128, 128], F32)
            nc.sync.dma_start(IDENT[:], ident[:])
            IDENTB = constp.tile([128, 128], BF16)
            nc.sync.dma_start(IDENTB[:], identb[:])
            EPS5 = constp.tile([128, 1], F32)
            nc.vector.memset(EPS5[:], 1e-5)
            ONES1 = constp.tile([1, 128], F32)
            nc.vector.memset(ONES1[:], 1.0)
            Z8 = constp.tile([HPC, Q], F32)
            nc.vector.memset(Z8[:], 0.0)
            hstate = constp.tile([128, CPC], F32, name="hstate")
            hstate_bf = constp.tile([128, CPC], BF16, name="hstate_bf")
            DBC = constp.tile([128, CPC], F32, name="DBC")
            with tc.tile_pool(name="pc0", bufs=1, space="PSUM") as pc0:
                dbcp = pc0.tile([128, CPC], F32)
                nc.tensor.matmul(dbcp[:], ONES1[:], DROW[:])
                nc.vector.tensor_copy(DBC[:], dbcp[:])

            # ---- phase 1 ----
            with (
                tc.tile_pool(name="p1", bufs=1) as p1,
                tc.tile_pool(name="pA", bufs=2, space="PSUM") as pA,
                tc.tile_pool(name="pSB", bufs=1, space="PSUM") as pSB,
                tc.tile_pool(name="pY", bufs=1, space="PSUM") as pY,
                tc.tile_pool(name="pI", bufs=2, space="PSUM") as pI,
                tc.tile_pool(name="pT", bufs=2, space="PSUM") as pT,
            ):
                for b in range(BB):
                    nc.vector.memset(hstate[:], 0.0)
                    nc.vector.memset(hstate_bf[:], 0.0)
                    xbcbuf_prev = None
                    for blk in range(HTOK // TC):
                        tok0 = b * HTOK + blk * TC
                        # ---- A: stats + quant + transpose, per subtile ----
                        invs = p1.tile([128, NSUB], F32, tag="invs", bufs=2)
                        isrow = p1.tile([1, TC], F32, tag="isrow", bufs=1)
                        qT = p1.tile([128, NK1 * TC], BF16, tag="qT", bufs=1)
                        for tt in range(NSUB):
                            Xin = p1.tile([128, DIM], F32, tag="Xin", bufs=2)
                            nc.sync.dma_start(
                                Xin[:], hid[tok0 + tt * 128: tok0 + (tt + 1) * 128, :])
                            G = p1.tile([128, DIM], F32, tag="G", bufs=2)
                            sh = p1.tile([128, 1], F32, tag="sh", bufs=2)
                            nc.vector.scalar_tensor_tensor(
                                G[:], Xin[:], 1.0, NW[:], ALU.mult, ALU.mult,
                                accum_out=sh[:])
                            scr = p1.tile([128, DIM], BF16, tag="scr", bufs=2)
                            shh = p1.tile([128, 1], F32, tag="shh", bufs=2)
                            nc.scalar.activation(scr[:], G[:], AF.Square,
                                                 accum_out=shh[:])
                            m = p1.tile([128, 1], F32, tag="m", bufs=2)
                            nc.vector.tensor_scalar(m[:], sh[:], 1.0 / DIM,
                                                    None, ALU.mult)
                            hmax = p1.tile([128, 1], F32, tag="hmax", bufs=2)
                            hmin = p1.tile([128, 1], F32, tag="hmin", bufs=2)
                            nc.vector.tensor_reduce(out=hmax[:], in_=G[:],
                                                    op=ALU.max, axis=AX.X)
                            nc.vector.tensor_reduce(out=hmin[:], in_=G[:],
                                                    op=ALU.min, axis=AX.X)
                            d1 = p1.tile([128, 1], F32, tag="d1", bufs=2)
                            nc.vector.tensor_scalar(d1[:], hmax[:], m[:],
                                                    None, ALU.subtract)
                            d2 = p1.tile([128, 1], F32, tag="d2", bufs=2)
                            nc.vector.tensor_scalar(d2[:], hmin[:], -1.0, m[:],
                                                    ALU.mult, ALU.add)
                            mab = p1.tile([128, 1], F32, tag="mab", bufs=2)
                            nc.vector.tensor_tensor(mab[:], d1[:], d2[:], ALU.max)
                            mm = p1.tile([128, 1], F32, tag="mm", bufs=2)
                            nc.vector.tensor_tensor(mm[:], m[:], m[:], ALU.mult)
                            vr = p1.tile([128, 1], F32, tag="vr", bufs=2)
                            nc.vector.scalar_tensor_tensor(
                                vr[:], shh[:], 1.0 / DIM, mm[:],
                                ALU.mult, ALU.subtract)
                            lv = p1.tile([128, 1], F32, tag="vre", bufs=2, name="lv")
                            nc.scalar.activation(lv[:], vr[:], AF.Ln, bias=EPS5[:])
                            rv = p1.tile([128, 1], F32, tag="rv", bufs=2)
                            nc.scalar.activation(rv[:], lv[:], AF.Exp, scale=-0.5)
                            u = p1.tile([128, 1], F32, tag="u", bufs=2)
                            nc.vector.tensor_tensor(u[:], mab[:], rv[:], ALU.mult)
                            t2 = p1.tile([128, 1], F32, tag="t2", bufs=2)
                            nc.vector.tensor_scalar(t2[:], u[:], 1e-5,
                                                    None, ALU.max)
                            nc.vector.tensor_scalar(invs[:, tt:tt + 1], t2[:],
                                                    1.0 / 127.0, None, ALU.mult)
                            rt = p1.tile([128, 1], F32, tag="rt", bufs=2)
                            nc.vector.reciprocal(rt[:], t2[:])
                            a1 = p1.tile([128, 1], F32, tag="a1", bufs=2)
                            nc.vector.tensor_scalar(a1[:], rt[:], 127.0, rv[:],
                                                    ALU.mult, ALU.mult)
                            ma = p1.tile([128, 1], F32, tag="ma", bufs=2)
                            nc.vector.tensor_tensor(ma[:], m[:], a1[:], ALU.mult)
                            b1t = p1.tile([128, 1], F32, tag="b1t", bufs=2)
                            nc.vector.tensor_scalar(b1t[:], ma[:], -1.0, None,
                                                    ALU.mult)
                            T2 = p1.tile([128, DIM], F32, tag="Xin", bufs=2,
                                         name="T2")
                            nc.vector.tensor_scalar(T2[:], G[:], a1[:], b1t[:],
                                                    ALU.mult, ALU.add)
                            qb = p1.tile([128, DIM], BF16, tag="scr", bufs=2,
                                         name="qb")
                            nc.vector.tensor_scalar(qb[:], T2[:], MAGIC, MAGIC,
                                                    ALU.add, ALU.subtract)
                            nc.sync.dma_start(isrow[0:1, tt * 128:(tt + 1) * 128],
                                              invs[:, tt:tt + 1])
                            if QT_ON_PE:
                                for d4 in range(NK1 // 4):
                                    pq = pT.tile([128, 512], BF16, tag="pq")
                                    for j in range(4):
                                        dd = d4 * 4 + j
                                        nc.tensor.transpose(
                                            pq[:, j * 128:(j + 1) * 128],
                                            qb[:, dd * 128:(dd + 1) * 128],
                                            IDENTB[:])
                                    dst = bass.AP(
                                        tensor=qT.tensor,
                                        offset=qT.offset + (d4 * 4) * TC + tt * 128,
                                        ap=[list(qT.ap[0]), [TC, 4], [1, 128]])
                                    nc.gpsimd.tensor_copy(dst, pq[:])
                            else:
                                eng = nc.sync if tt % 2 == 0 else nc.scalar
                                for dd in range(NK1):
                                    eng.dma_start_transpose(
                                        qT[:, dd * TC + tt * 128: dd * TC + (tt + 1) * 128],
                                        qb[:, dd * 128:(dd + 1) * 128])
                        # SB broadcast of per-token dequant scales
                        psb_ = pA.tile([128, TC], F32, tag="mA")
                        nc.tensor.matmul(psb_[:], ONES1[:], isrow[:])
                        SB = p1.tile([128, TC], F32, tag="SBt", bufs=2)
                        nc.scalar.copy(SB[:], psb_[:])

                        # ---- D: z matmuls (token-major) ----
                        sz = p1.tile([128, NSUB * CPC], BF16, tag="sz", bufs=2)
                        for tt in range(NSUB):
                            pz = pA.tile([128, CPC], F32, tag="mA")
                            for kk in range(NK1):
                                nc.tensor.matmul(
                                    pz[:],
                                    qT[:, kk * TC + tt * 128: kk * TC + (tt + 1) * 128],
                                    W1S[:, kk * W1COLS + 896: (kk + 1) * W1COLS],
                                    start=(kk == 0), stop=(kk == NK1 - 1))
                            nc.scalar.activation(sz[:, tt * CPC:(tt + 1) * CPC], pz[:],
                                                 AF.Silu, scale=invs[:, tt:tt + 1])
                            if DEBUG_TAPS:
                                nc.sync.dma_start(
                                    taps["tap_z"][tok0 + tt * 128: tok0 + (tt + 1) * 128, :],
                                    sz[:, tt * CPC:(tt + 1) * CPC])

                        # ---- E: xBC + dt matmuls (channel-major) ----
                        xbcbuf = p1.tile([128, 6 * (TC + 3)], BF16, tag="xbcbuf",
                                         bufs=2)
                        dtraw = p1.tile([HPC, TC], F32, tag="dtraw", bufs=2)
                        for cbk in range(7):
                            px = pA.tile([128, TC], F32, tag="mA")
                            for kk in range(NK1):
                                nc.tensor.matmul(
                                    px[:],
                                    W1S[:, kk * W1COLS + cbk * 128: kk * W1COLS + (cbk + 1) * 128],
                                    qT[:, kk * TC:(kk + 1) * TC],
                                    start=(kk == 0), stop=(kk == NK1 - 1))
                            if cbk < 6:
                                nc.vector.tensor_tensor(
                                    xbcbuf[:, cbk * (TC + 3) + 3: (cbk + 1) * (TC + 3)],
                                    px[:], SB[:], ALU.mult)
                            else:
                                nc.vector.tensor_tensor(dtraw[:], px[0:HPC, :],
                                                        SB[0:HPC, :], ALU.mult)
                        for ct in range(6):
                            h0 = xbcbuf[:, ct * (TC + 3): ct * (TC + 3) + 3]
                            if blk == 0:
                                nc.vector.memset(h0, 0.0)
                            else:
                                nc.vector.tensor_copy(
                                    h0,
                                    xbcbuf_prev[:, ct * (TC + 3) + TC: (ct + 1) * (TC + 3)])
                        xbcbuf_prev = xbcbuf

                        # ---- F: conv + silu ----
                        xcm = p1.tile([128, 4 * TC], BF16, tag="xcm", bufs=2)
                        bbf = p1.tile([128, TC], BF16, tag="bbf", bufs=2)
                        cbf = p1.tile([128, TC], BF16, tag="cbf", bufs=2)
                        for ct in range(6):
                            conv = p1.tile([128, TC], F32, tag="conv", bufs=2)
                            base = ct * (TC + 3)
                            eng = nc.vector
                            eng.tensor_scalar(conv[:], xbcbuf[:, base: base + TC],
                                              CW[:, ct * D_CONV: ct * D_CONV + 1],
                                              None, ALU.mult)
                            for k in range(1, D_CONV):
                                eng.scalar_tensor_tensor(
                                    conv[:], xbcbuf[:, base + k: base + k + TC],
                                    CW[:, ct * D_CONV + k: ct * D_CONV + k + 1],
                                    conv[:], ALU.mult, ALU.add)
                            dst = (xcm[:, ct * TC:(ct + 1) * TC] if ct < 4
                                   else (bbf[:] if ct == 4 else cbf[:]))
                            nc.scalar.activation(dst, conv[:], AF.Silu,
                                                 bias=CBt[:, ct:ct + 1])

                        if DEBUG_TAPS:
                            for ct in range(6):
                                srctile = (xcm[:, ct * TC:(ct + 1) * TC] if ct < 4
                                           else (bbf[:] if ct == 4 else cbf[:]))
                                nc.sync.dma_start(
                                    taps["tap_conv"][ct * 128:(ct + 1) * 128,
                                                     tok0: tok0 + TC], srctile)
                                nc.sync.dma_start(
                                    taps["tap_xbc"][ct * 128:(ct + 1) * 128,
                                                    tok0: tok0 + TC],
                                    xbcbuf[:, ct * (TC + 3) + 3: (ct + 1) * (TC + 3)])

                        # ---- G: dt pipeline ----
                        dts = p1.tile([HPC, TC], F32, tag="dts", bufs=2)
                        t_ab = p1.tile([HPC, TC], F32, tag="dtw", bufs=2, name="tab")
                        nc.scalar.activation(t_ab[:], dtraw[:], AF.Abs, bias=DTB[:])
                        t_e = p1.tile([HPC, TC], F32, tag="dtw", bufs=2, name="te")
                        nc.scalar.activation(t_e[:], t_ab[:], AF.Exp, scale=-1.0)
                        t_l = p1.tile([HPC, TC], F32, tag="dtw", bufs=2, name="tl")
                        nc.scalar.activation(t_l[:], t_e[:], AF.Ln, bias=1.0)
                        t_r = p1.tile([HPC, TC], F32, tag="dtw", bufs=2, name="tr")
                        nc.scalar.activation(t_r[:], dtraw[:], AF.Relu, bias=DTB[:])
                        nc.vector.tensor_tensor(dts[:], t_l[:], t_r[:], ALU.add)
                        if DEBUG_TAPS:
                            nc.sync.dma_start(taps["tap_dt"][:, tok0: tok0 + TC], dts[:])
                        av = p1.tile([HPC, TC], F32, tag="dtraw", bufs=2, name="av")
                        nc.vector.tensor_scalar(av[:], dts[:], ACO[:], None, ALU.mult)

                        # ---- H: scan chunks ----
                        for cq in range(TC // Q):
                            csl = slice(cq * Q, (cq + 1) * Q)
                            STD = p1.tile([96, Q], F32, tag="STD", bufs=2)
                            nc.vector.tensor_tensor_scan(
                                STD[0:8, :], av[:, csl], Z8[:], 0.0,
                                ALU.add, ALU.add)
                            nc.vector.tensor_scalar(
                                STD[32:40, :], STD[0:8, :], -1.0,
                                STD[0:8, Q - 1:Q], ALU.mult, ALU.add)
                            nc.vector.tensor_copy(STD[64:72, :], dts[:, csl])
                            srow = p1.tile([1, HPC * Q], F32, tag="srow", bufs=1)
                            nc.scalar.dma_start(row_view(srow[:], HPC, Q),
                                                STD[0:8, :])
                            pstd = pT.tile([128, 512], F32, tag="mT")
                            nc.tensor.transpose(pstd[:, 0:96], STD[:],
                                                IDENT[0:96, 0:96])
                            ee = p1.tile([128, 16], F32, tag="ee", bufs=2)
                            nc.scalar.activation(ee[:, 0:8], pstd[:, 0:8], AF.Exp)
                            nc.scalar.activation(ee[:, 8:16], pstd[:, 32:40], AF.Exp)
                            dtsT = p1.tile([128, HPC], F32, tag="dtsT", bufs=2)
                            nc.vector.tensor_copy(dtsT[:], pstd[:, 64:72])
                            STs = p1.tile([128, HPC], F32, tag="STs", bufs=2)
                            nc.scalar.copy(STs[:], pstd[:, 0:8])
                            eSQ = p1.tile([128, HPC], F32, tag="eSQ", bufs=2)
                            nc.vector.tensor_tensor(eSQ[:], ee[:, 0:8], ee[:, 8:16],
                                                    ALU.mult)
                            de = p1.tile([128, HPC], BF16, tag="de", bufs=2)
                            nc.vector.tensor_tensor(de[:], dtsT[:], ee[:, 8:16],
                                                    ALU.mult)
                            x_tm = p1.tile([128, CPC], BF16, tag="x_tm", bufs=2)
                            for ct in range(4):
                                nc.scalar.dma_start_transpose(
                                    x_tm[:, ct * 128:(ct + 1) * 128],
                                    xcm[:, ct * TC + cq * Q: ct * TC + (cq + 1) * Q])
                            btm = p1.tile([128, 128], BF16, tag="btm", bufs=2)
                            nc.scalar.dma_start_transpose(btm[:], bbf[:, csl])
                            ppg = pT.tile([128, 512], F32, tag="mT")
                            nc.tensor.matmul(ppg[:, 0:128], bbf[:, csl], cbf[:, csl])
                            PG = p1.tile([128, 128], BF16, tag="PG", bufs=2)
                            nc.scalar.copy(PG[:], ppg[:, 0:128])
                            Mx = p1.tile([128, HPC * Q], BF16, tag="Mx", bufs=2)
                            for hh2 in range(2):
                                ppsb = pSB.tile([128, 512], F32, tag="mS")
                                nc.tensor.matmul(
                                    ppsb[:], ONES1[:],
                                    srow[0:1, hh2 * 512:(hh2 + 1) * 512],
                                    start=True, stop=False)
                                nc.tensor.matmul(ppsb[:], IDENTB[:], MASKT[:],
                                                 start=False, stop=True)
                                Dm = p1.tile([128, 512], F32, tag="Dm", bufs=2)
                                nc.vector.tensor_tensor(
                                    Dm[:], ppsb[:],
                                    bcast(STs[:, hh2 * 4: hh2 * 4 + 4], 4, 1, Q),
                                    ALU.subtract)
                                nc.scalar.activation(
                                    Mx[:, hh2 * 512:(hh2 + 1) * 512], Dm[:], AF.Exp)
                            wm1 = p1.tile([128, HPC * Q], BF16, tag="wm1", bufs=2)
                            nc.vector.tensor_tensor(
                                wm1[:], Mx[:], bcast(dtsT[:], HPC, 1, Q), ALU.mult)
                            Wm = p1.tile([128, HPC * Q], BF16, tag="wm2", bufs=2, name="Wm")
                            nc.vector.tensor_tensor(
                                Wm[:], wm1[:], block_rep(PG[:], HPC, Q), ALU.mult)
                            X3 = p1.tile([128, CPC], BF16, tag="X3", bufs=2)
                            nc.vector.tensor_tensor(
                                X3[:], x_tm[:], bcast(de[:], HPC, 1, HEADDIM),
                                ALU.mult)
                            pyi = pY.tile([128, CPC], F32, tag="mY")
                            for hh in range(HPC):
                                nc.tensor.matmul(
                                    pyi[:, hh * HEADDIM:(hh + 1) * HEADDIM],
                                    Wm[:, hh * Q:(hh + 1) * Q],
                                    x_tm[:, hh * HEADDIM:(hh + 1) * HEADDIM],
                                    start=True, stop=True)
                            ppye = pI.tile([128, CPC], F32, tag="mI")
                            nc.tensor.matmul(ppye[:], cbf[:, csl], hstate_bf[:])
                            ppd = pI.tile([128, CPC], F32, tag="mI")
                            nc.tensor.matmul(ppd[:], btm[:], X3[:])
                            # state update (serial spine, vector)
                            nc.vector.tensor_tensor(
                                hstate[:], hstate[:],
                                bcast(eSQ[:], HPC, 1, HEADDIM), ALU.mult)
                            nc.vector.tensor_tensor(hstate[:], hstate[:], ppd[:],
                                                    ALU.add)
                            nc.vector.tensor_copy(hstate_bf[:], hstate[:])
                            # y assembly
                            yt = p1.tile([128, CPC], F32, tag="yt", bufs=2)
                            nc.vector.tensor_tensor(
                                yt[:], ppye[:], bcast(ee[:, 0:8], HPC, 1, HEADDIM),
                                ALU.mult)
                            y2 = p1.tile([128, CPC], F32, tag="yt", bufs=2, name="y2")
                            nc.vector.tensor_tensor(y2[:], yt[:], pyi[:], ALU.add)
                            dx = p1.tile([128, CPC], BF16, tag="dx", bufs=2)
                            nc.vector.tensor_tensor(dx[:], x_tm[:], DBC[:], ALU.mult)
                            y3 = p1.tile([128, CPC], F32, tag="yt", bufs=2, name="y3")
                            nc.vector.tensor_tensor(y3[:], y2[:], dx[:], ALU.add)
                            y4 = p1.tile([128, CPC], BF16, tag="y4", bufs=2)
                            nc.vector.tensor_tensor(
                                y4[:], y3[:], sz[:, cq * CPC:(cq + 1) * CPC],
                                ALU.mult)
                            nc.sync.dma_start(
                                a2a_in[b][blk * TC + cq * Q: blk * TC + (cq + 1) * Q, :],
                                y4[:])
                            if DEBUG_TAPS:
                                nc.sync.dma_start(
                                    taps["tap_y"][tok0 + cq * Q: tok0 + (cq + 1) * Q, :],
                                    y4[:])
                    # batch b fully written -> AllToAll (overlaps next batch)
                    nc.gpsimd.collective_compute(
                        "AllToAll", ALU.bypass,
                        replica_groups=[list(range(NCORES))],
                        ins=[a2a_in[b][:]], outs=[a2a_out[b][:]])

            # ---- phase 2: out-norm + quant + out_proj, per 128-token subtile ----
            with (
                tc.tile_pool(name="p2", bufs=1) as p2,
                tc.tile_pool(name="pO", bufs=1, space="PSUM") as pO,
            ):
                ONW = p2.tile([128, D_INNER], F32, tag="ONW")
                nc.sync.dma_start(ONW[:], onwb[:])
                for half in range(2):
                    for tt in range(2):
                        st = half * 2 + tt   # global subtile index 0..3
                        Y2 = p2.tile([128, D_INNER], BF16, tag="Y2", bufs=2)
                        for j in range(NCORES):
                            nc.sync.dma_start(
                                Y2[:, j * CPC:(j + 1) * CPC],
                                a2a_out[half][j * TPH + tt * 128: j * TPH + (tt + 1) * 128, :])
                        G = p2.tile([128, D_INNER], F32, tag="G2", bufs=2)
                        sh = p2.tile([128, 1], F32, tag="sh2", bufs=2)
                        nc.vector.scalar_tensor_tensor(
                            G[:], Y2[:], 1.0, ONW[:], ALU.mult, ALU.mult,
                            accum_out=sh[:])
                        scr = p2.tile([128, D_INNER], BF16, tag="scr2", bufs=2)
                        shh = p2.tile([128, 1], F32, tag="shh2", bufs=2)
                        nc.scalar.activation(scr[:], G[:], AF.Square,
                                             accum_out=shh[:])
                        m = p2.tile([128, 1], F32, tag="m2", bufs=2)
                        nc.vector.tensor_scalar(m[:], sh[:], 1.0 / D_INNER,
                                                None, ALU.mult)
                        hmax = p2.tile([128, 1], F32, tag="hmax2", bufs=2)
                        hmin = p2.tile([128, 1], F32, tag="hmin2", bufs=2)
                        nc.vector.tensor_reduce(out=hmax[:], in_=G[:],
                                                op=ALU.max, axis=AX.X)
                        nc.vector.tensor_reduce(out=hmin[:], in_=G[:],
                                                op=ALU.min, axis=AX.X)
                        d1 = p2.tile([128, 1], F32, tag="d12", bufs=2)
                        nc.vector.tensor_scalar(d1[:], hmax[:], m[:],
                                                None, ALU.subtract)
                        d2 = p2.tile([128, 1], F32, tag="d22", bufs=2)
                        nc.vector.tensor_scalar(d2[:], hmin[:], -1.0, m[:],
                                                ALU.mult, ALU.add)
                        mab = p2.tile([128, 1], F32, tag="mab2", bufs=2)
                        nc.vector.tensor_tensor(mab[:], d1[:], d2[:], ALU.max)
                        mm = p2.tile([128, 1], F32, tag="mm2", bufs=2)
                        nc.vector.tensor_tensor(mm[:], m[:], m[:], ALU.mult)
                        vr = p2.tile([128, 1], F32, tag="vr2", bufs=2)
                        nc.vector.scalar_tensor_tensor(
                            vr[:], shh[:], 1.0 / D_INNER, mm[:],
                            ALU.mult, ALU.subtract)
                        lv = p2.tile([128, 1], F32, tag="rec2", bufs=2, name="lv2")
                        nc.scalar.activation(lv[:], vr[:], AF.Ln, bias=EPS5[:])
                        rv = p2.tile([128, 1], F32, tag="rv2", bufs=2)
                        nc.scalar.activation(rv[:], lv[:], AF.Exp, scale=-0.5)
                        t1 = p2.tile([128, 1], F32, tag="t12", bufs=2)
                        nc.vector.tensor_tensor(t1[:], mab[:], rv[:], ALU.mult)
                        t2 = p2.tile([128, 1], F32, tag="t22", bufs=2)
                        nc.vector.tensor_scalar(t2[:], t1[:], 1e-5, None, ALU.max)
                        invs2 = p2.tile([128, 1], F32, tag="invs2", bufs=2)
                        nc.vector.tensor_scalar(invs2[:], t2[:], 1.0 / 127.0,
                                                None, ALU.mult)
                        rt = p2.tile([128, 1], F32, tag="rt2", bufs=2)
                        nc.vector.reciprocal(rt[:], t2[:])
                        a4 = p2.tile([128, 1], F32, tag="a42", bufs=2)
                        nc.vector.tensor_scalar(a4[:], rt[:], 127.0, rv[:],
                                                ALU.mult, ALU.mult)
                        ma = p2.tile([128, 1], F32, tag="ma2", bufs=2)
                        nc.vector.tensor_tensor(ma[:], m[:], a4[:], ALU.mult)
                        b4 = p2.tile([128, 1], F32, tag="b42", bufs=2)
                        nc.vector.tensor_scalar(b4[:], ma[:], -1.0, None,
                                                ALU.mult)
                        T2 = p2.tile([128, D_INNER], F32, tag="G2", bufs=2, name="T2p2")
                        nc.vector.tensor_scalar(T2[:], G[:], a4[:], b4[:],
                                                ALU.mult, ALU.add)
                        qb = p2.tile([128, D_INNER], BF16, tag="scr2", bufs=2, name="qbp2")
                        nc.vector.tensor_scalar(qb[:], T2[:], MAGIC, MAGIC,
                                                ALU.add, ALU.subtract)
                        qT2 = p2.tile([128, NK2 * 128], BF16, tag="qT2", bufs=2)
                        eng = nc.sync if tt % 2 == 0 else nc.scalar
                        for dd in range(NK2):
                            eng.dma_start_transpose(
                                qT2[:, dd * 128:(dd + 1) * 128],
                                qb[:, dd * 128:(dd + 1) * 128])
                        # out_proj: accumulate [128t, 2048] over 32 k-tiles
                        po = [pO.tile([128, 512], F32, tag=f"po{c}", name=f"po{c}_{st}")
                              for c in range(4)]
                        for kk in range(NK2):
                            w2sb = p2.tile([128, DIM], BF16, tag="w2sb", bufs=2)
                            nc.sync.dma_start(w2sb[:], w2t[kk * 128:(kk + 1) * 128, :])
                            for c in range(4):
                                nc.tensor.matmul(
                                    po[c][:],
                                    qT2[:, kk * 128:(kk + 1) * 128],
                                    w2sb[:, c * 512:(c + 1) * 512],
                                    start=(kk == 0), stop=(kk == NK2 - 1))
                        for c in range(4):
                            hidt = p2.tile([128, 512], F32, tag="hidt", bufs=2)
                            nc.sync.dma_start(
                                hidt[:],
                                hid2[st * 128:(st + 1) * 128, c * 512:(c + 1) * 512])
                            ot = p2.tile([128, 512], F32, tag="ot", bufs=2)
                            nc.vector.scalar_tensor_tensor(
                                ot[:], po[c][:], invs2[:], hidt[:],
                                ALU.mult, ALU.add)
                            nc.sync.dma_start(
                                out[st * 128:(st + 1) * 128, c * 512:(c + 1) * 512],
                                ot[:])

    nc.compile()
    return nc


_CACHE = {}


def _prep_inputs(inputs):
    hid = np.ascontiguousarray(
        np.asarray(inputs["hidden_states"], np.float32).reshape(NTOK, DIM))
    w1 = np.asarray(inputs["in_proj_w"], np.float32)
    w2 = np.asarray(inputs["out_proj_w"], np.float32)

    def wquant(w):
        scale = max(np.float32(np.mean(np.abs(w), dtype=np.float32)),
                    np.float32(1e-5))
        ws = w / scale
        return np.clip(np.where(ws >= 0, np.floor(ws + 0.5), np.ceil(ws - 0.5)),
                       -1.0, 1.0).astype(np.float32)

    q1 = wquant(w1)
    q2 = wquant(w2)
    conv_w = np.asarray(inputs["conv_w"], np.float32)[:, 0, :]
    conv_b = np.asarray(inputs["conv_b"], np.float32)
    A = -np.exp(np.asarray(inputs["A_log"], np.float32))
    Dv = np.asarray(inputs["D"], np.float32)
    dtb = np.asarray(inputs["dt_bias"], np.float32)
    nw = np.asarray(inputs["norm_w"], np.float32)
    onw = np.asarray(inputs["out_norm_w"], np.float32)

    nwb = np.ascontiguousarray(np.broadcast_to(nw[None, :], (128, DIM)))
    onwb = np.ascontiguousarray(np.broadcast_to(onw[None, :], (128, D_INNER)))
    w2t = np.ascontiguousarray(q2.T.astype(ml_dtypes.bfloat16))
    ii, jj = np.meshgrid(np.arange(128), np.arange(128), indexing="ij")
    mask1 = np.where(ii > jj, np.float32(-1e30), np.float32(0.0))
    maskt = np.ascontiguousarray(
        np.tile(mask1, (1, 4)).astype(ml_dtypes.bfloat16))
    identity = np.eye(128, dtype=np.float32)
    identb = np.eye(128, dtype=np.float32).astype(ml_dtypes.bfloat16)

    in_maps = []
    for k in range(NCORES):
        wk = np.concatenate([
            q1[D_INNER + CPC * k: D_INNER + CPC * (k + 1)],        # x
            q1[2 * D_INNER: 2 * D_INNER + D_STATE],                # B
            q1[2 * D_INNER + D_STATE: 2 * D_INNER + 2 * D_STATE],  # C
            np.concatenate([q1[2 * D_INNER + 2 * D_STATE + HPC * k:
                               2 * D_INNER + 2 * D_STATE + HPC * (k + 1)],
                            np.zeros((128 - HPC, DIM), np.float32)], axis=0),
            q1[CPC * k: CPC * (k + 1)],                            # z
        ], axis=0)
        w1tk = np.ascontiguousarray(wk.T.astype(ml_dtypes.bfloat16))
        cwk = np.concatenate([conv_w[CPC * k: CPC * (k + 1)],
                              conv_w[D_INNER: D_INNER + D_STATE],
                              conv_w[D_INNER + D_STATE:]], axis=0)
        cbk = np.concatenate([conv_b[CPC * k: CPC * (k + 1)],
                              conv_b[D_INNER: D_INNER + D_STATE],
                              conv_b[D_INNER + D_STATE:]])[:, None]
        hid2_k = np.concatenate([hid[TPH * k: TPH * (k + 1)],
                                 hid[HTOK + TPH * k: HTOK + TPH * (k + 1)]],
                                axis=0)
        in_maps.append({
            "hid": hid,
            "hid2": np.ascontiguousarray(hid2_k),
            "w1t": w1tk,
            "w2t": w2t,
            "nwb": nwb,
            "onwb": onwb,
            "cw": np.ascontiguousarray(cwk),
            "cb": np.ascontiguousarray(cbk),
            "dtb": np.ascontiguousarray(dtb[HPC * k: HPC * (k + 1)][:, None]),
            "acoef": np.ascontiguousarray(A[HPC * k: HPC * (k + 1)][:, None]),
            "maskt": maskt,
            "drow": np.ascontiguousarray(
                np.repeat(Dv[HPC * k: HPC * (k + 1)], HEADDIM)[None, :]),
            "ident": identity,
            "identb": np.ascontiguousarray(identb),
        })
    return in_maps


def kernel(**inputs):
    if "nc" not in _CACHE:
        _CACHE["nc"] = build_bass()
    nc = _CACHE["nc"]
    in_maps = _prep_inputs(inputs)
    res = bass_utils.run_bass_kernel_spmd(nc, in_maps, core_ids=list(range(NCORES)))
    _CACHE["last_results"] = res
    outp = np.empty((NTOK, DIM), np.float32)
    for k in range(NCORES):
        r = res.results[k]["out"]
        outp[TPH * k: TPH * (k + 1)] = r[0:TPH]
        outp[HTOK + TPH * k: HTOK + TPH * (k + 1)] = r[TPH:TPC]
    return outp.reshape(BB, L, DIM).astype(np.float32)


# revision 24
# speedup vs baseline: 1.6637x; 1.4147x over previous
"""BitMambaBlock Trainium2 kernel — 8-core SPMD, v2.

Sharding: phase 1 (in_proj + conv + SSD scan) head-sharded (8 heads/core over
all 4096 tokens); phase 2 (out_proj) token-sharded (512 tokens/core, 256 from
each batch) after a 2-stage bf16 AllToAll (batch-0 a2a overlaps batch-1
phase-1 compute).

Numerics: bitlinear matmuls use exact integer bf16 operands (quant ints in
[-127,127], ternary weights) with fp32 PSUM accumulation. The rmsnorm before
each bitlinear folds away (layernorm_noaffine is invariant to per-token
positive scaling), so LN stats are computed directly on x*norm_w. SSD scan
uses the chunked masked-segsum form, chunk=128, bf16 matmul operands, fp32
state; dt and the D*x skip-term are folded into the intra-chunk weight matrix.
"""

import sys
import types
import numpy as np
import ml_dtypes

for _p in ("/opt/trn_rl_repo", "/root/.axon_site/_ro/trn_rl_repo"):
    if _p not in sys.path:
        sys.path.insert(0, _p)

try:
    import antenv

    if "antenv.axon_hooks" not in sys.modules:
        _mod = types.ModuleType("antenv.axon_hooks")
        _HOOK = [None]
        _mod.set_axon_ntff_profile_hook = lambda h: _HOOK.__setitem__(0, h)
        _mod.get_axon_ntff_profile_hook = lambda: _HOOK[0]
        sys.modules["antenv.axon_hooks"] = _mod
        antenv.axon_hooks = _mod
        try:
            from trn_agent_boot.trn_boot import _ntff_profile_via_ctypes

            _mod.set_axon_ntff_profile_hook(
                _ntff_profile_via_ctypes("/opt/axon/libaxon_pjrt.so")
            )
        except Exception:
            pass
except Exception:
    pass

import concourse.bass as bass
import concourse.bacc as bacc
import concourse.mybir as mybir
from concourse.tile import TileContext
from concourse import bass_utils

F32 = mybir.dt.float32
BF16 = mybir.dt.bfloat16
AF = mybir.ActivationFunctionType
ALU = mybir.AluOpType
AX = mybir.AxisListType

DIM = 2048
D_STATE = 128
D_CONV = 4
HEADDIM = 64
D_INNER = 4096
NHEADS = 64
D_IN_PROJ = 8512
CONV_DIM = 4352
BB, L = 2, 2048
NTOK = BB * L              # 4096
NCORES = 8
HPC = NHEADS // NCORES     # 8 heads per core
CPC = HPC * HEADDIM        # 512 d_inner channels per core
TPC = NTOK // NCORES       # 512 tokens per core (phase 2)
HTOK = NTOK // 2           # 2048 tokens per batch
TPH = TPC // 2             # 256 tokens per core per half
TC = 512                   # phase-1 block
NSUB = TC // 128           # 4 subtiles per block
Q = 128                    # scan chunk
MAGIC = float(np.float32(12582912.0))
NK1 = DIM // 128           # 16 k-tiles for in_proj
W1COLS = 1408              # x(512) B(128) C(128) dtpad(128) z(512); z at 896
NK2 = D_INNER // 128       # 32 k-tiles for out_proj

QT_ON_PE = True           # qb transposes: False=DMA queues, True=TensorE
DEBUG_TAPS = False


def bcast(ap, n_outer, stride_outer, rep):
    """[P, n_outer(+)] -> [P, n_outer, rep] view (each col repeated rep x)."""
    return bass.AP(tensor=ap.tensor, offset=ap.offset,
                   ap=[list(ap.ap[0]), [stride_outer, n_outer], [0, rep]])


def block_rep(ap, nrep, ncols):
    """[P, ncols] -> [P, nrep, ncols] view (whole block repeated nrep x)."""
    st = ap.ap[1][0]
    return bass.AP(tensor=ap.tensor, offset=ap.offset,
                   ap=[list(ap.ap[0]), [0, nrep], [st, ncols]])


def row_view(tile_ap, n_outer, inner):
    """[n_outer, inner] partition-major tile viewed as [1, n_outer, inner]."""
    return bass.AP(tensor=tile_ap.tensor, offset=tile_ap.offset,
                   ap=[[tile_ap.ap[0][0], 1], [inner, n_outer], [1, inner]])


def build_bass():
    nc = bacc.Bacc(None, target_bir_lowering=False, num_devices=NCORES)

    hid = nc.dram_tensor("hid", [NTOK, DIM], F32, kind="ExternalInput")
    hid2 = nc.dram_tensor("hid2", [TPC, DIM], F32, kind="ExternalInput")
    w1t = nc.dram_tensor("w1t", [DIM, W1COLS], BF16, kind="ExternalInput")
    w2t = nc.dram_tensor("w2t", [D_INNER, DIM], BF16, kind="ExternalInput")
    nwb = nc.dram_tensor("nwb", [128, DIM], F32, kind="ExternalInput")
    onwb = nc.dram_tensor("onwb", [128, D_INNER], F32, kind="ExternalInput")
    cw = nc.dram_tensor("cw", [768, D_CONV], F32, kind="ExternalInput")
    cb = nc.dram_tensor("cb", [768, 1], F32, kind="ExternalInput")
    dtb = nc.dram_tensor("dtb", [HPC, 1], F32, kind="ExternalInput")
    acoef = nc.dram_tensor("acoef", [HPC, 1], F32, kind="ExternalInput")
    maskt = nc.dram_tensor("maskt", [128, 512], BF16, kind="ExternalInput")
    drow = nc.dram_tensor("drow", [1, CPC], F32, kind="ExternalInput")
    ident = nc.dram_tensor("ident", [128, 128], F32, kind="ExternalInput")
    identb = nc.dram_tensor("identb", [128, 128], BF16, kind="ExternalInput")

    out = nc.dram_tensor("out", [TPC, DIM], F32, kind="ExternalOutput")
    taps = {}
    if DEBUG_TAPS:
        taps["tap_z"] = nc.dram_tensor("tap_z", [NTOK, CPC], BF16, kind="ExternalOutput")
        taps["tap_conv"] = nc.dram_tensor("tap_conv", [768, NTOK], BF16, kind="ExternalOutput")
        taps["tap_dt"] = nc.dram_tensor("tap_dt", [HPC, NTOK], F32, kind="ExternalOutput")
        taps["tap_y"] = nc.dram_tensor("tap_y", [NTOK, CPC], BF16, kind="ExternalOutput")
        taps["tap_xbc"] = nc.dram_tensor("tap_xbc", [768, NTOK], BF16, kind="ExternalOutput")

    with TileContext(nc) as tc:
        with (
            tc.tile_pool(name="const", bufs=1) as constp,
            tc.tile_pool(name="dram", bufs=1, space="DRAM") as dram,
        ):
            a2a_in = [dram.tile([HTOK, CPC], BF16, name=f"a2ai{i}") for i in (0, 1)]
            a2a_out = [dram.tile([HTOK, CPC], BF16, name=f"a2ao{i}") for i in (0, 1)]

            # ---- resident constants ----
            W1S = constp.tile([128, NK1 * W1COLS], BF16)
            for kk in range(NK1):
                nc.sync.dma_start(W1S[:, kk * W1COLS:(kk + 1) * W1COLS],
                                  w1t[kk * 128:(kk + 1) * 128, :])
            NW = constp.tile([128, DIM], F32)
            nc.sync.dma_start(NW[:], nwb[:])
            CW = constp.tile([128, 6 * D_CONV], F32)
            CBt = constp.tile([128, 6], F32)
            for ct in range(6):
                nc.sync.dma_start(CW[:, ct * D_CONV:(ct + 1) * D_CONV],
                                  cw[ct * 128:(ct + 1) * 128, :])
                nc.sync.dma_start(CBt[:, ct:ct + 1], cb[ct * 128:(ct + 1) * 128, :])
            DTB = constp.tile([HPC, 1], F32)
            nc.sync.dma_start(DTB[:], dtb[:])
            ACO = constp.tile([HPC, 1], F32)
            nc.sync.dma_start(ACO[:], acoef[:])
            MASKT = constp.tile([128, 512], BF16)
            nc.sync.dma_start(MASKT[:], maskt[:])
            DROW = constp.tile([1, CPC], F32)
            nc.sync.dma_start(DROW[:], drow[:])
            IDENT = constp.tile([128, 128], F32)
            nc.sync.dma_start(IDENT[:], ident[:])
            IDENTB = constp.tile([128, 128], BF16)
            nc.sync.dma_start(IDENTB[:], identb[:])
            EPS5 = constp.tile([128, 1], F32)
            nc.vector.memset(EPS5[:], 1e-5)
            ONES1 = constp.tile([1, 128], F32)
            nc.vector.memset(ONES1[:], 1.0)
            Z8 = constp.tile([HPC, Q], F32)
            nc.vector.memset(Z8[:], 0.0)
            hstate = constp.tile([128, CPC], F32, name="hstate")
            hstate_bf = constp.tile([128, CPC], BF16, name="hstate_bf")
            DBC = constp.tile([128, CPC], F32, name="DBC")
            with tc.tile_pool(name="pc0", bufs=1, space="PSUM") as pc0:
                dbcp = pc0.tile([128, CPC], F32)
                nc.tensor.matmul(dbcp[:], ONES1[:], DROW[:])
                nc.vector.tensor_copy(DBC[:], dbcp[:])

            # ---- phase 1 ----
            with (
                tc.tile_pool(name="p1", bufs=1) as p1,
                tc.tile_pool(name="pA", bufs=2, space="PSUM") as pA,
                tc.tile_pool(name="pSB", bufs=1, space="PSUM") as pSB,
                tc.tile_pool(name="pY", bufs=1, space="PSUM") as pY,
                tc.tile_pool(name="pI", bufs=1, space="PSUM") as pI,
                tc.tile_pool(name="pT", bufs=2, space="PSUM") as pT,
            ):
                for b in range(BB):
                    nc.vector.memset(hstate[:], 0.0)
                    nc.vector.memset(hstate_bf[:], 0.0)
                    xbcbuf_prev = None
                    for blk in range(HTOK // TC):
                        tok0 = b * HTOK + blk * TC
                        # ---- A: stats + quant + transpose, per subtile ----
                        invs = p1.tile([128, NSUB], F32, tag="invs", bufs=2)
                        isrow = p1.tile([1, TC], F32, tag="isrow", bufs=1)
                        qT = p1.tile([128, NK1 * TC], BF16, tag="qT", bufs=1)
                        for tt in range(NSUB):
                            Xin = p1.tile([128, DIM], F32, tag="Xin", bufs=2)
                            nc.sync.dma_start(
                                Xin[:], hid[tok0 + tt * 128: tok0 + (tt + 1) * 128, :])
                            G = p1.tile([128, DIM], F32, tag="G", bufs=2)
                            sh = p1.tile([128, 1], F32, tag="sh", bufs=2)
                            nc.vector.scalar_tensor_tensor(
                                G[:], Xin[:], 1.0, NW[:], ALU.mult, ALU.mult,
                                accum_out=sh[:])
                            scr = p1.tile([128, DIM], BF16, tag="scr", bufs=2)
                            shh = p1.tile([128, 1], F32, tag="shh", bufs=2)
                            nc.scalar.activation(scr[:], G[:], AF.Square,
                                                 accum_out=shh[:])
                            m = p1.tile([128, 1], F32, tag="m", bufs=2)
                            nc.vector.tensor_scalar(m[:], sh[:], 1.0 / DIM,
                                                    None, ALU.mult)
                            hmax = p1.tile([128, 1], F32, tag="hmax", bufs=2)
                            hmin = p1.tile([128, 1], F32, tag="hmin", bufs=2)
                            nc.vector.tensor_reduce(out=hmax[:], in_=G[:],
                                                    op=ALU.max, axis=AX.X)
                            nc.vector.tensor_reduce(out=hmin[:], in_=G[:],
                                                    op=ALU.min, axis=AX.X)
                            d1 = p1.tile([128, 1], F32, tag="d1", bufs=2)
                            nc.vector.tensor_scalar(d1[:], hmax[:], m[:],
                                                    None, ALU.subtract)
                            d2 = p1.tile([128, 1], F32, tag="d2", bufs=2)
                            nc.vector.tensor_scalar(d2[:], hmin[:], -1.0, m[:],
                                                    ALU.mult, ALU.add)
                            mab = p1.tile([128, 1], F32, tag="mab", bufs=2)
                            nc.vector.tensor_tensor(mab[:], d1[:], d2[:], ALU.max)
                            mm = p1.tile([128, 1], F32, tag="mm", bufs=2)
                            nc.vector.tensor_tensor(mm[:], m[:], m[:], ALU.mult)
                            vr = p1.tile([128, 1], F32, tag="vr", bufs=2)
                            nc.vector.scalar_tensor_tensor(
                                vr[:], shh[:], 1.0 / DIM, mm[:],
                                ALU.mult, ALU.subtract)
                            lv = p1.tile([128, 1], F32, tag="vre", bufs=2, name="lv")
                            nc.scalar.activation(lv[:], vr[:], AF.Ln, bias=EPS5[:])
                            rv = p1.tile([128, 1], F32, tag="rv", bufs=2)
                            nc.scalar.activation(rv[:], lv[:], AF.Exp, scale=-0.5)
                            u = p1.tile([128, 1], F32, tag="u", bufs=2)
                            nc.vector.tensor_tensor(u[:], mab[:], rv[:], ALU.mult)
                            t2 = p1.tile([128, 1], F32, tag="t2", bufs=2)
                            nc.vector.tensor_scalar(t2[:], u[:], 1e-5,
                                                    None, ALU.max)
                            nc.vector.tensor_scalar(invs[:, tt:tt + 1], t2[:],
                                                    1.0 / 127.0, None, ALU.mult)
                            rt = p1.tile([128, 1], F32, tag="rt", bufs=2)
                            nc.vector.reciprocal(rt[:], t2[:])
                            a1 = p1.tile([128, 1], F32, tag="a1", bufs=2)
                            nc.vector.tensor_scalar(a1[:], rt[:], 127.0, rv[:],
                                                    ALU.mult, ALU.mult)
                            ma = p1.tile([128, 1], F32, tag="ma", bufs=2)
                            nc.vector.tensor_tensor(ma[:], m[:], a1[:], ALU.mult)
                            b1t = p1.tile([128, 1], F32, tag="b1t", bufs=2)
                            nc.vector.tensor_scalar(b1t[:], ma[:], -1.0, None,
                                                    ALU.mult)
                            T2 = p1.tile([128, DIM], F32, tag="Xin", bufs=2,
                                         name="T2")
                            nc.vector.tensor_scalar(T2[:], G[:], a1[:], b1t[:],
                                                    ALU.mult, ALU.add)
                            qb = p1.tile([128, DIM], BF16, tag="scr", bufs=2,
                                         name="qb")
                            nc.vector.tensor_scalar(qb[:], T2[:], MAGIC, MAGIC,
                                                    ALU.add, ALU.subtract)
                            nc.sync.dma_start(isrow[0:1, tt * 128:(tt + 1) * 128],
                                              invs[:, tt:tt + 1])
                            if QT_ON_PE:
                                for d4 in range(NK1 // 4):
                                    pq = pT.tile([128, 512], BF16, tag="pq", bufs=1)
                                    for j in range(4):
                                        dd = d4 * 4 + j
                                        nc.tensor.transpose(
                                            pq[:, j * 128:(j + 1) * 128],
                                            qb[:, dd * 128:(dd + 1) * 128],
                                            IDENTB[:])
                                    dst = bass.AP(
                                        tensor=qT.tensor,
                                        offset=qT.offset + (d4 * 4) * TC + tt * 128,
                                        ap=[list(qT.ap[0]), [TC, 4], [1, 128]])
                                    nc.vector.tensor_copy(dst, pq[:])
                            else:
                                eng = nc.sync if tt % 2 == 0 else nc.scalar
                                for dd in range(NK1):
                                    eng.dma_start_transpose(
                                        qT[:, dd * TC + tt * 128: dd * TC + (tt + 1) * 128],
                                        qb[:, dd * 128:(dd + 1) * 128])
                        # SB broadcast of per-token dequant scales
                        psb_ = pA.tile([128, TC], F32, tag="mA")
                        nc.tensor.matmul(psb_[:], ONES1[:], isrow[:])
                        SB = p1.tile([128, TC], F32, tag="SBt", bufs=2)
                        nc.scalar.copy(SB[:], psb_[:])

                        # ---- D: z matmuls (token-major) ----
                        sz = p1.tile([128, NSUB * CPC], BF16, tag="sz", bufs=2)
                        for tt in range(NSUB):
                            pz = pA.tile([128, CPC], F32, tag="mA")
                            for kk in range(NK1):
                                nc.tensor.matmul(
                                    pz[:],
                                    qT[:, kk * TC + tt * 128: kk * TC + (tt + 1) * 128],
                                    W1S[:, kk * W1COLS + 896: (kk + 1) * W1COLS],
                                    start=(kk == 0), stop=(kk == NK1 - 1))
                            nc.scalar.activation(sz[:, tt * CPC:(tt + 1) * CPC], pz[:],
                                                 AF.Silu, scale=invs[:, tt:tt + 1])
                            if DEBUG_TAPS:
                                nc.sync.dma_start(
                                    taps["tap_z"][tok0 + tt * 128: tok0 + (tt + 1) * 128, :],
                                    sz[:, tt * CPC:(tt + 1) * CPC])

                        # ---- E: xBC + dt matmuls (channel-major) ----
                        xbcbuf = p1.tile([128, 6 * (TC + 3)], BF16, tag="xbcbuf",
                                         bufs=2)
                        dtraw = p1.tile([HPC, TC], F32, tag="dtraw", bufs=2)
                        for cbk in range(7):
                            px = pA.tile([128, TC], F32, tag="mA")
                            for kk in range(NK1):
                                nc.tensor.matmul(
                                    px[:],
                                    W1S[:, kk * W1COLS + cbk * 128: kk * W1COLS + (cbk + 1) * 128],
                                    qT[:, kk * TC:(kk + 1) * TC],
                                    start=(kk == 0), stop=(kk == NK1 - 1))
                            if cbk < 6:
                                nc.vector.tensor_tensor(
                                    xbcbuf[:, cbk * (TC + 3) + 3: (cbk + 1) * (TC + 3)],
                                    px[:], SB[:], ALU.mult)
                            else:
                                nc.vector.tensor_tensor(dtraw[:], px[0:HPC, :],
                                                        SB[0:HPC, :], ALU.mult)
                        for ct in range(6):
                            h0 = xbcbuf[:, ct * (TC + 3): ct * (TC + 3) + 3]
                            if blk == 0:
                                nc.vector.memset(h0, 0.0)
                            else:
                                nc.vector.tensor_copy(
                                    h0,
                                    xbcbuf_prev[:, ct * (TC + 3) + TC: (ct + 1) * (TC + 3)])
                        xbcbuf_prev = xbcbuf

                        # ---- F: conv + silu ----
                        xcm = p1.tile([128, 4 * TC], BF16, tag="xcm", bufs=2)
                        bbf = p1.tile([128, TC], BF16, tag="bbf", bufs=2)
                        cbf = p1.tile([128, TC], BF16, tag="cbf", bufs=2)
                        for ct in range(6):
                            conv = p1.tile([128, TC], F32, tag="conv", bufs=2)
                            base = ct * (TC + 3)
                            eng = nc.vector
                            eng.tensor_scalar(conv[:], xbcbuf[:, base: base + TC],
                                              CW[:, ct * D_CONV: ct * D_CONV + 1],
                                              None, ALU.mult)
                            for k in range(1, D_CONV):
                                eng.scalar_tensor_tensor(
                                    conv[:], xbcbuf[:, base + k: base + k + TC],
                                    CW[:, ct * D_CONV + k: ct * D_CONV + k + 1],
                                    conv[:], ALU.mult, ALU.add)
                            dst = (xcm[:, ct * TC:(ct + 1) * TC] if ct < 4
                                   else (bbf[:] if ct == 4 else cbf[:]))
                            nc.scalar.activation(dst, conv[:], AF.Silu,
                                                 bias=CBt[:, ct:ct + 1])

                        if DEBUG_TAPS:
                            for ct in range(6):
                                srctile = (xcm[:, ct * TC:(ct + 1) * TC] if ct < 4
                                           else (bbf[:] if ct == 4 else cbf[:]))
                                nc.sync.dma_start(
                                    taps["tap_conv"][ct * 128:(ct + 1) * 128,
                                                     tok0: tok0 + TC], srctile)
                                nc.sync.dma_start(
                                    taps["tap_xbc"][ct * 128:(ct + 1) * 128,
                                                    tok0: tok0 + TC],
                                    xbcbuf[:, ct * (TC + 3) + 3: (ct + 1) * (TC + 3)])

                        # ---- G: dt pipeline ----
                        dts = p1.tile([HPC, TC], F32, tag="dts", bufs=2)
                        t_ab = p1.tile([HPC, TC], F32, tag="dtw", bufs=2, name="tab")
                        nc.scalar.activation(t_ab[:], dtraw[:], AF.Abs, bias=DTB[:])
                        t_e = p1.tile([HPC, TC], F32, tag="dtw", bufs=2, name="te")
                        nc.scalar.activation(t_e[:], t_ab[:], AF.Exp, scale=-1.0)
                        t_l = p1.tile([HPC, TC], F32, tag="dtw", bufs=2, name="tl")
                        nc.scalar.activation(t_l[:], t_e[:], AF.Ln, bias=1.0)
                        t_r = p1.tile([HPC, TC], F32, tag="dtw", bufs=2, name="tr")
                        nc.scalar.activation(t_r[:], dtraw[:], AF.Relu, bias=DTB[:])
                        nc.vector.tensor_tensor(dts[:], t_l[:], t_r[:], ALU.add)
                        if DEBUG_TAPS:
                            nc.sync.dma_start(taps["tap_dt"][:, tok0: tok0 + TC], dts[:])
                        av = p1.tile([HPC, TC], F32, tag="dtraw", bufs=2, name="av")
                        nc.vector.tensor_scalar(av[:], dts[:], ACO[:], None, ALU.mult)

                        # ---- H: scan chunks ----
                        for cq in range(TC // Q):
                            csl = slice(cq * Q, (cq + 1) * Q)
                            STD = p1.tile([96, Q], F32, tag="STD", bufs=2)
                            nc.vector.tensor_tensor_scan(
                                STD[0:8, :], av[:, csl], Z8[:], 0.0,
                                ALU.add, ALU.add)
                            nc.vector.tensor_scalar(
                                STD[32:40, :], STD[0:8, :], -1.0,
                                STD[0:8, Q - 1:Q], ALU.mult, ALU.add)
                            nc.vector.tensor_copy(STD[64:72, :], dts[:, csl])
                            srow = p1.tile([1, HPC * Q], F32, tag="srow", bufs=1)
                            nc.scalar.dma_start(row_view(srow[:], HPC, Q),
                                                STD[0:8, :])
                            pstd = pT.tile([128, 512], F32, tag="mT")
                            nc.tensor.transpose(pstd[:, 0:96], STD[:],
                                                IDENT[0:96, 0:96])
                            ee = p1.tile([128, 16], F32, tag="ee", bufs=2)
                            nc.scalar.activation(ee[:, 0:8], pstd[:, 0:8], AF.Exp)
                            nc.scalar.activation(ee[:, 8:16], pstd[:, 32:40], AF.Exp)
                            dtsT = p1.tile([128, HPC], F32, tag="dtsT", bufs=2)
                            nc.vector.tensor_copy(dtsT[:], pstd[:, 64:72])
                            STs = p1.tile([128, HPC], F32, tag="STs", bufs=2)
                            nc.scalar.copy(STs[:], pstd[:, 0:8])
                            eSQ = p1.tile([128, HPC], F32, tag="eSQ", bufs=2)
                            nc.vector.tensor_tensor(eSQ[:], ee[:, 0:8], ee[:, 8:16],
                                                    ALU.mult)
                            de = p1.tile([128, HPC], BF16, tag="de", bufs=2)
                            nc.vector.tensor_tensor(de[:], dtsT[:], ee[:, 8:16],
                                                    ALU.mult)
                            x_tm = p1.tile([128, CPC], BF16, tag="x_tm", bufs=2)
                            for ct in range(4):
                                nc.scalar.dma_start_transpose(
                                    x_tm[:, ct * 128:(ct + 1) * 128],
                                    xcm[:, ct * TC + cq * Q: ct * TC + (cq + 1) * Q])
                            btm = p1.tile([128, 128], BF16, tag="btm", bufs=2)
                            nc.scalar.dma_start_transpose(btm[:], bbf[:, csl])
                            ppg = pT.tile([128, 512], F32, tag="mT")
                            nc.tensor.matmul(ppg[:, 0:128], bbf[:, csl], cbf[:, csl])
                            PG = p1.tile([128, 128], BF16, tag="PG", bufs=2)
                            nc.scalar.copy(PG[:], ppg[:, 0:128])
                            Mx = p1.tile([128, HPC * Q], BF16, tag="Mx", bufs=2)
                            for hh2 in range(2):
                                ppsb = pSB.tile([128, 512], F32, tag="mS")
                                nc.tensor.matmul(
                                    ppsb[:], ONES1[:],
                                    srow[0:1, hh2 * 512:(hh2 + 1) * 512],
                                    start=True, stop=False)
                                nc.tensor.matmul(ppsb[:], IDENTB[:], MASKT[:],
                                                 start=False, stop=True)
                                Dm = p1.tile([128, 512], F32, tag="Dm", bufs=2)
                                nc.vector.tensor_tensor(
                                    Dm[:], ppsb[:],
                                    bcast(STs[:, hh2 * 4: hh2 * 4 + 4], 4, 1, Q),
                                    ALU.subtract)
                                nc.scalar.activation(
                                    Mx[:, hh2 * 512:(hh2 + 1) * 512], Dm[:], AF.Exp)
                            wm1 = p1.tile([128, HPC * Q], BF16, tag="wm1", bufs=2)
                            nc.vector.tensor_tensor(
                                wm1[:], Mx[:], bcast(dtsT[:], HPC, 1, Q), ALU.mult)
                            Wm = p1.tile([128, HPC * Q], BF16, tag="wm2", bufs=2, name="Wm")
                            nc.vector.tensor_tensor(
                                Wm[:], wm1[:], block_rep(PG[:], HPC, Q), ALU.mult)
                            X3 = p1.tile([128, CPC], BF16, tag="X3", bufs=2)
                            nc.vector.tensor_tensor(
                                X3[:], x_tm[:], bcast(de[:], HPC, 1, HEADDIM),
                                ALU.mult)
                            pyi = pY.tile([128, CPC], F32, tag="mY")
                            for hh in range(HPC):
                                nc.tensor.matmul(
                                    pyi[:, hh * HEADDIM:(hh + 1) * HEADDIM],
                                    Wm[:, hh * Q:(hh + 1) * Q],
                                    x_tm[:, hh * HEADDIM:(hh + 1) * HEADDIM],
                                    start=True, stop=True)
                            ppye = pI.tile([128, CPC], F32, tag="mI")
                            nc.tensor.matmul(ppye[:], cbf[:, csl], hstate_bf[:])
                            ppd = pI.tile([128, CPC], F32, tag="mI")
                            nc.tensor.matmul(ppd[:], btm[:], X3[:])
                            # state update (serial spine, vector)
                            nc.vector.tensor_tensor(
                                hstate[:], hstate[:],
                                bcast(eSQ[:], HPC, 1, HEADDIM), ALU.mult)
                            nc.vector.tensor_tensor(hstate[:], hstate[:], ppd[:],
                                                    ALU.add)
                            nc.vector.tensor_copy(hstate_bf[:], hstate[:])
                            # y assembly
                            yt = p1.tile([128, CPC], F32, tag="yt", bufs=2)
                            nc.vector.tensor_tensor(
                                yt[:], ppye[:], bcast(ee[:, 0:8], HPC, 1, HEADDIM),
                                ALU.mult)
                            y2 = p1.tile([128, CPC], F32, tag="yt", bufs=2, name="y2")
                            nc.vector.tensor_tensor(y2[:], yt[:], pyi[:], ALU.add)
                            dx = p1.tile([128, CPC], BF16, tag="dx", bufs=2)
                            nc.vector.tensor_tensor(dx[:], x_tm[:], DBC[:], ALU.mult)
                            y3 = p1.tile([128, CPC], F32, tag="yt", bufs=2, name="y3")
                            nc.vector.tensor_tensor(y3[:], y2[:], dx[:], ALU.add)
                            y4 = p1.tile([128, CPC], BF16, tag="y4", bufs=2)
                            nc.vector.tensor_tensor(
                                y4[:], y3[:], sz[:, cq * CPC:(cq + 1) * CPC],
                                ALU.mult)
                            nc.sync.dma_start(
                                a2a_in[b][blk * TC + cq * Q: blk * TC + (cq + 1) * Q, :],
                                y4[:])
                            if DEBUG_TAPS:
                                nc.sync.dma_start(
                                    taps["tap_y"][tok0 + cq * Q: tok0 + (cq + 1) * Q, :],
                                    y4[:])
                    # batch b fully written -> AllToAll (overlaps next batch)
                    nc.gpsimd.collective_compute(
                        "AllToAll", ALU.bypass,
                        replica_groups=[list(range(NCORES))],
                        ins=[a2a_in[b][:]], outs=[a2a_out[b][:]])

            # ---- phase 2: out-norm + quant + out_proj, per 128-token subtile ----
            with (
                tc.tile_pool(name="p2", bufs=1) as p2,
                tc.tile_pool(name="pO", bufs=1, space="PSUM") as pO,
            ):
                ONW = p2.tile([128, D_INNER], F32, tag="ONW")
                nc.sync.dma_start(ONW[:], onwb[:])
                for half in range(2):
                    for tt in range(2):
                        st = half * 2 + tt   # global subtile index 0..3
                        Y2 = p2.tile([128, D_INNER], BF16, tag="Y2", bufs=2)
                        for j in range(NCORES):
                            nc.sync.dma_start(
                                Y2[:, j * CPC:(j + 1) * CPC],
                                a2a_out[half][j * TPH + tt * 128: j * TPH + (tt + 1) * 128, :])
                        G = p2.tile([128, D_INNER], F32, tag="G2", bufs=2)
                        sh = p2.tile([128, 1], F32, tag="sh2", bufs=2)
                        nc.vector.scalar_tensor_tensor(
                            G[:], Y2[:], 1.0, ONW[:], ALU.mult, ALU.mult,
                            accum_out=sh[:])
                        scr = p2.tile([128, D_INNER], BF16, tag="scr2", bufs=2)
                        shh = p2.tile([128, 1], F32, tag="shh2", bufs=2)
                        nc.scalar.activation(scr[:], G[:], AF.Square,
                                             accum_out=shh[:])
                        m = p2.tile([128, 1], F32, tag="m2", bufs=2)
                        nc.vector.tensor_scalar(m[:], sh[:], 1.0 / D_INNER,
                                                None, ALU.mult)
                        hmax = p2.tile([128, 1], F32, tag="hmax2", bufs=2)
                        hmin = p2.tile([128, 1], F32, tag="hmin2", bufs=2)
                        nc.vector.tensor_reduce(out=hmax[:], in_=G[:],
                                                op=ALU.max, axis=AX.X)
                        nc.vector.tensor_reduce(out=hmin[:], in_=G[:],
                                                op=ALU.min, axis=AX.X)
                        d1 = p2.tile([128, 1], F32, tag="d12", bufs=2)
                        nc.vector.tensor_scalar(d1[:], hmax[:], m[:],
                                                None, ALU.subtract)
                        d2 = p2.tile([128, 1], F32, tag="d22", bufs=2)
                        nc.vector.tensor_scalar(d2[:], hmin[:], -1.0, m[:],
                                                ALU.mult, ALU.add)
                        mab = p2.tile([128, 1], F32, tag="mab2", bufs=2)
                        nc.vector.tensor_tensor(mab[:], d1[:], d2[:], ALU.max)
                        mm = p2.tile([128, 1], F32, tag="mm2", bufs=2)
                        nc.vector.tensor_tensor(mm[:], m[:], m[:], ALU.mult)
                        vr = p2.tile([128, 1], F32, tag="vr2", bufs=2)
                        nc.vector.scalar_tensor_tensor(
                            vr[:], shh[:], 1.0 / D_INNER, mm[:],
                            ALU.mult, ALU.subtract)
                        lv = p2.tile([128, 1], F32, tag="rec2", bufs=2, name="lv2")
                        nc.scalar.activation(lv[:], vr[:], AF.Ln, bias=EPS5[:])
                        rv = p2.tile([128, 1], F32, tag="rv2", bufs=2)
                        nc.scalar.activation(rv[:], lv[:], AF.Exp, scale=-0.5)
                        t1 = p2.tile([128, 1], F32, tag="t12", bufs=2)
                        nc.vector.tensor_tensor(t1[:], mab[:], rv[:], ALU.mult)
                        t2 = p2.tile([128, 1], F32, tag="t22", bufs=2)
                        nc.vector.tensor_scalar(t2[:], t1[:], 1e-5, None, ALU.max)
                        invs2 = p2.tile([128, 1], F32, tag="invs2", bufs=2)
                        nc.vector.tensor_scalar(invs2[:], t2[:], 1.0 / 127.0,
                                                None, ALU.mult)
                        rt = p2.tile([128, 1], F32, tag="rt2", bufs=2)
                        nc.vector.reciprocal(rt[:], t2[:])
                        a4 = p2.tile([128, 1], F32, tag="a42", bufs=2)
                        nc.vector.tensor_scalar(a4[:], rt[:], 127.0, rv[:],
                                                ALU.mult, ALU.mult)
                        ma = p2.tile([128, 1], F32, tag="ma2", bufs=2)
                        nc.vector.tensor_tensor(ma[:], m[:], a4[:], ALU.mult)
                        b4 = p2.tile([128, 1], F32, tag="b42", bufs=2)
                        nc.vector.tensor_scalar(b4[:], ma[:], -1.0, None,
                                                ALU.mult)
                        T2 = p2.tile([128, D_INNER], F32, tag="G2", bufs=2, name="T2p2")
                        nc.vector.tensor_scalar(T2[:], G[:], a4[:], b4[:],
                                                ALU.mult, ALU.add)
                        qb = p2.tile([128, D_INNER], BF16, tag="scr2", bufs=2, name="qbp2")
                        nc.vector.tensor_scalar(qb[:], T2[:], MAGIC, MAGIC,
                                                ALU.add, ALU.subtract)
                        qT2 = p2.tile([128, NK2 * 128], BF16, tag="qT2", bufs=2)
                        for d4 in range(NK2 // 4):
                            pq = pO.tile([128, 512], BF16, tag="pq2", bufs=2)
                            for j in range(4):
                                dd = d4 * 4 + j
                                nc.tensor.transpose(
                                    pq[:, j * 128:(j + 1) * 128],
                                    qb[:, dd * 128:(dd + 1) * 128],
                                    IDENTB[:])
                            nc.vector.tensor_copy(
                                qT2[:, d4 * 512:(d4 + 1) * 512], pq[:])
                        # out_proj: accumulate [128t, 2048] over 32 k-tiles
                        po = [pO.tile([128, 512], F32, tag=f"po{c}", name=f"po{c}_{st}")
                              for c in range(4)]
                        for kk in range(NK2):
                            w2sb = p2.tile([128, DIM], BF16, tag="w2sb", bufs=2)
                            nc.sync.dma_start(w2sb[:], w2t[kk * 128:(kk + 1) * 128, :])
                            for c in range(4):
                                nc.tensor.matmul(
                                    po[c][:],
                                    qT2[:, kk * 128:(kk + 1) * 128],
                                    w2sb[:, c * 512:(c + 1) * 512],
                                    start=(kk == 0), stop=(kk == NK2 - 1))
                        for c in range(4):
                            hidt = p2.tile([128, 512], F32, tag="hidt", bufs=2)
                            nc.sync.dma_start(
                                hidt[:],
                                hid2[st * 128:(st + 1) * 128, c * 512:(c + 1) * 512])
                            ot = p2.tile([128, 512], F32, tag="ot", bufs=2)
                            nc.vector.scalar_tensor_tensor(
                                ot[:], po[c][:], invs2[:], hidt[:],
                                ALU.mult, ALU.add)
                            nc.sync.dma_start(
                                out[st * 128:(st + 1) * 128, c * 512:(c + 1) * 512],
                                ot[:])

    nc.compile()
    return nc


_CACHE = {}


def _prep_inputs(inputs):
    hid = np.ascontiguousarray(
        np.asarray(inputs["hidden_states"], np.float32).reshape(NTOK, DIM))
    w1 = np.asarray(inputs["in_proj_w"], np.float32)
    w2 = np.asarray(inputs["out_proj_w"], np.float32)

    def wquant(w):
        scale = max(np.float32(np.mean(np.abs(w), dtype=np.float32)),
                    np.float32(1e-5))
        ws = w / scale
        return np.clip(np.where(ws >= 0, np.floor(ws + 0.5), np.ceil(ws - 0.5)),
                       -1.0, 1.0).astype(np.float32)

    q1 = wquant(w1)
    q2 = wquant(w2)
    conv_w = np.asarray(inputs["conv_w"], np.float32)[:, 0, :]
    conv_b = np.asarray(inputs["conv_b"], np.float32)
    A = -np.exp(np.asarray(inputs["A_log"], np.float32))
    Dv = np.asarray(inputs["D"], np.float32)
    dtb = np.asarray(inputs["dt_bias"], np.float32)
    nw = np.asarray(inputs["norm_w"], np.float32)
    onw = np.asarray(inputs["out_norm_w"], np.float32)

    nwb = np.ascontiguousarray(np.broadcast_to(nw[None, :], (128, DIM)))
    onwb = np.ascontiguousarray(np.broadcast_to(onw[None, :], (128, D_INNER)))
    w2t = np.ascontiguousarray(q2.T.astype(ml_dtypes.bfloat16))
    ii, jj = np.meshgrid(np.arange(128), np.arange(128), indexing="ij")
    mask1 = np.where(ii > jj, np.float32(-1e30), np.float32(0.0))
    maskt = np.ascontiguousarray(
        np.tile(mask1, (1, 4)).astype(ml_dtypes.bfloat16))
    identity = np.eye(128, dtype=np.float32)
    identb = np.eye(128, dtype=np.float32).astype(ml_dtypes.bfloat16)

    in_maps = []
    for k in range(NCORES):
        wk = np.concatenate([
            q1[D_INNER + CPC * k: D_INNER + CPC * (k + 1)],        # x
            q1[2 * D_INNER: 2 * D_INNER + D_STATE],                # B
            q1[2 * D_INNER + D_STATE: 2 * D_INNER + 2 * D_STATE],  # C
            np.concatenate([q1[2 * D_INNER + 2 * D_STATE + HPC * k:
                               2 * D_INNER + 2 * D_STATE + HPC * (k + 1)],
                            np.zeros((128 - HPC, DIM), np.float32)], axis=0),
            q1[CPC * k: CPC * (k + 1)],                            # z
        ], axis=0)
        w1tk = np.ascontiguousarray(wk.T.astype(ml_dtypes.bfloat16))
        cwk = np.concatenate([conv_w[CPC * k: CPC * (k + 1)],
                              conv_w[D_INNER: D_INNER + D_STATE],
                              conv_w[D_INNER + D_STATE:]], axis=0)
        cbk = np.concatenate([conv_b[CPC * k: CPC * (k + 1)],
                              conv_b[D_INNER: D_INNER + D_STATE],
                              conv_b[D_INNER + D_STATE:]])[:, None]
        hid2_k = np.concatenate([hid[TPH * k: TPH * (k + 1)],
                                 hid[HTOK + TPH * k: HTOK + TPH * (k + 1)]],
                                axis=0)
        in_maps.append({
            "hid": hid,
            "hid2": np.ascontiguousarray(hid2_k),
            "w1t": w1tk,
            "w2t": w2t,
            "nwb": nwb,
            "onwb": onwb,
            "cw": np.ascontiguousarray(cwk),
            "cb": np.ascontiguousarray(cbk),
            "dtb": np.ascontiguousarray(dtb[HPC * k: HPC * (k + 1)][:, None]),
            "acoef": np.ascontiguousarray(A[HPC * k: HPC * (k + 1)][:, None]),
            "maskt": maskt,
            "drow": np.ascontiguousarray(
                np.repeat(Dv[HPC * k: HPC * (k + 1)], HEADDIM)[None, :]),
            "ident": identity,
            "identb": np.ascontiguousarray(identb),
        })
    return in_maps


def kernel(**inputs):
    if "nc" not in _CACHE:
        _CACHE["nc"] = build_bass()
    nc = _CACHE["nc"]
    in_maps = _prep_inputs(inputs)
    res = bass_utils.run_bass_kernel_spmd(nc, in_maps, core_ids=list(range(NCORES)))
    _CACHE["last_results"] = res
    outp = np.empty((NTOK, DIM), np.float32)
    for k in range(NCORES):
        r = res.results[k]["out"]
        outp[TPH * k: TPH * (k + 1)] = r[0:TPH]
        outp[HTOK + TPH * k: HTOK + TPH * (k + 1)] = r[TPH:TPC]
    return outp.reshape(BB, L, DIM).astype(np.float32)
